# revision 1
# baseline (speedup 1.0000x reference)
"""Bond-centered tensor-moment descriptor kernel for Trainium2 (8 NeuronCores).

Strategy: edges are sharded 8 ways; every core gets the full (relaid-out)
atom-descriptor table and gathers its edge endpoints with indirect DMA.
The Clebsch-Gordan tensor product is computed as: build Z[e,(f,a,b)] =
sh_a(u)*rad_f(r)*y_b,f with per-partition-scalar ops, transpose Z to
feature-partitions with TensorE, then one stationary matmul per f-pair
whose weights fold CG coefficients and tp_weights.
"""
import math
import numpy as np

import concourse.bass as bass
import concourse.tile as tile
from concourse import mybir
from concourse.bass import AP
from concourse.bass_utils import run_bass_kernel_spmd
from concourse.masks import make_identity
from concourse.tile import TileContext, ScopedClock

# ----------------------------------------------------------------------------
# Problem constants (hardcoded per contract)
# ----------------------------------------------------------------------------
CUTOFF = 5.0
MAX_BASIS_DEG = 2
MAX_DEG = 4
N_ATOMS = 20000
N_EDGES = 50000
F = 16
N_CORES = 8

NSH = (MAX_BASIS_DEG + 1) ** 2        # 9 spherical-harmonic components
NB = (MAX_DEG + 1) ** 2               # 25 atom-feature m-slots
BPAD = 26                             # b padded for 4-byte alignment of a*BPAD
NC_OUT = 2 * NB                       # 50 output (parity, c) slots
ABLK = NSH * BPAD                     # 234 Z-columns per f
FPBLK = 512                           # padded Z-columns per f-pair (2*ABLK=468 -> 512)
ZCOLS = 8 * FPBLK                     # 4096
EPC = 6400                            # edges per core (padded from 6250)
EBLK = 128                            # edges per block
NBLK = EPC // EBLK                    # 50 blocks per core

PATHS = [(l1, l2, l3)
         for l1 in range(MAX_BASIS_DEG + 1)
         for l2 in range(MAX_DEG + 1)
         for l3 in range(abs(l1 - l2), min(l1 + l2, MAX_DEG) + 1)]

ZDT = mybir.dt.bfloat16               # Z / zT / W dtype (flip to float32 if precision demands)
ZNP = np.dtype("bfloat16") if False else None  # host cast handled via ml_dtypes below


# ----------------------------------------------------------------------------
# Clebsch-Gordan coefficients (host, numpy only)
# ----------------------------------------------------------------------------
def _fac(n):
    return math.factorial(n)


def _cg(j1, m1, j2, m2, j3, m3):
    if m1 + m2 != m3:
        return 0.0
    if j3 < abs(j1 - j2) or j3 > j1 + j2:
        return 0.0
    pre = math.sqrt((2 * j3 + 1) * _fac(j3 + j1 - j2) * _fac(j3 - j1 + j2)
                    * _fac(j1 + j2 - j3) / _fac(j1 + j2 + j3 + 1))
    pre *= math.sqrt(_fac(j3 + m3) * _fac(j3 - m3) * _fac(j1 - m1) * _fac(j1 + m1)
                     * _fac(j2 - m2) * _fac(j2 + m2))
    s = 0.0
    for k in range(max(0, j2 - j3 - m1, j1 - j3 + m2),
                   min(j1 + j2 - j3, j1 - m1, j2 + m2) + 1):
        s += (-1) ** k / (_fac(k) * _fac(j1 + j2 - j3 - k) * _fac(j1 - m1 - k)
                          * _fac(j2 + m2 - k) * _fac(j3 - j2 + m1 + k)
                          * _fac(j3 - j1 - m2 + k))
    return pre * s


def _umat(l):
    U = np.zeros((2 * l + 1, 2 * l + 1), dtype=np.complex128)
    s2 = 1.0 / np.sqrt(2.0)
    for m in range(-l, l + 1):
        if m > 0:
            U[m + l, m + l] = ((-1) ** m) * s2
            U[m + l, -m + l] = s2
        elif m == 0:
            U[l, l] = 1.0
        else:
            am = -m
            U[m + l, m + l] = 1j * s2
            U[m + l, am + l] = -1j * ((-1) ** am) * s2
    return U


def _real_cg(l1, l2, l3):
    C = np.zeros((2 * l1 + 1, 2 * l2 + 1, 2 * l3 + 1), dtype=np.complex128)
    for m1 in range(-l1, l1 + 1):
        for m2 in range(-l2, l2 + 1):
            m3 = m1 + m2
            if -l3 <= m3 <= l3:
                C[m1 + l1, m2 + l2, m3 + l3] = _cg(l1, m1, l2, m2, l3, m3)
    G = np.einsum('aA,bB,cC,ABC->abc', _umat(l1), _umat(l2),
                  np.conj(_umat(l3)), C)
    G = G.real if (l1 + l2 + l3) % 2 == 0 else G.imag
    return np.ascontiguousarray(G)


def _build_weight_tensor(tp_weights):
    """W[f, a, b, c, ] -> big [ZCOLS, NC_OUT] matrix in the Z-column order
    (f-major, then a, then padded b), entry = CG[a,b,c] * tp_weights[path, f]."""
    G_abc = np.zeros((NSH, NB, NC_OUT), dtype=np.float64)
    for p, (l1, l2, l3) in enumerate(PATHS):
        G = _real_cg(l1, l2, l3)
        par = (l1 + l2 + l3) % 2
        for ai in range(2 * l1 + 1):
            for bi in range(2 * l2 + 1):
                for ci in range(2 * l3 + 1):
                    v = G[ai, bi, ci]
                    if v != 0.0:
                        ga = l1 * l1 + ai
                        gb = l2 * l2 + bi
                        gc = par * NB + l3 * l3 + ci
                        G_abc[ga, gb, gc] = v
    # per-path tp weight lookup per (a,b,c) triple
    path_idx = {}
    for p, (l1, l2, l3) in enumerate(PATHS):
        path_idx[(l1, l2, l3)] = p
    l_of_a = [0, 1, 1, 1, 2, 2, 2, 2, 2]
    l_of_b = [int(np.sqrt(b)) for b in range(NB)]
    l_of_c = [int(np.sqrt(c % NB)) for c in range(NC_OUT)]

    W = np.zeros((F, NSH, BPAD, NC_OUT), dtype=np.float64)
    for ga in range(NSH):
        for gb in range(NB):
            nz = np.nonzero(G_abc[ga, gb])[0]
            if len(nz) == 0:
                continue
            for gc in nz:
                p = path_idx[(l_of_a[ga], l_of_b[gb], l_of_c[gc])]
                for f in range(F):
                    W[f, ga, gb, gc] = G_abc[ga, gb, gc] * float(tp_weights[p, f])
    W = W.reshape(F, ABLK, NC_OUT)
    # assemble per-f-pair stationaries [FPBLK, 2*NC_OUT] with f block-diag M
    out = np.zeros((8, FPBLK, 2 * NC_OUT), dtype=np.float64)
    for fp in range(8):
        for df in range(2):
            out[fp, df * ABLK:(df + 1) * ABLK, df::2] = W[2 * fp + df]
    return out.reshape(8 * FPBLK, 2 * NC_OUT)


# ----------------------------------------------------------------------------
# Device kernel builder
# ----------------------------------------------------------------------------
_NC_CACHE = {}


def _drain_and_barrier_patched(self, tick_clock, wait_clock):
    # this container's walrus supports only one sync-wait per CTRL
    nc = self.nc
    drain_inst = nc.sync.drain()
    wait_clock.add_sem_waits(drain_inst.ins,
                             ScopedClock({None: tick_clock.global_clock}))
    si = drain_inst.ins.sync_info
    waits = list(si.on_wait) if si else []
    if len(waits) > 1:
        drain_inst.ins.sync_info = mybir.SyncInfo(on_wait=[waits[0]],
                                                  on_update=list(si.on_update))
        for w in waits[1:]:
            d2 = nc.sync.drain()
            d2.ins.sync_info = mybir.SyncInfo(on_wait=[w], on_update=[])
    nc.all_engine_barrier()
    assert self.sems is not None
    popped = nc._tile_sem_poison_stack.pop()
    assert popped is self._sem_poison
    nc.clear_and_free_semaphores(list(self.sems.allocated().values()))
    nc.all_engine_barrier()


TileContext._drain_and_barrier = _drain_and_barrier_patched

# each f-pair owns exactly 4 aligned 128-row zT chunks
KBLK = FPBLK


def _kpieces(fp):
    return [(4 * fp + i, 0, 128) for i in range(4)]


def _split_multi_waits(nc):
    """This container's walrus supports one sync-wait per instruction; move
    extra waits onto injected same-engine NoOps placed just before."""
    for f in nc.m.functions:
        for bb in f.blocks:
            newl = []
            changed = False
            for inst in bb.instructions:
                si = inst.sync_info
                waits = list(si.on_wait) if si else []
                if len(waits) > 1:
                    changed = True
                    for k, w in enumerate(waits[:-1]):
                        nop = mybir.InstDrain(name=f"{inst.name}-sw{k}",
                                              ins=[], outs=[])
                        nop.engine = inst.engine
                        nop.sync_info = mybir.SyncInfo(on_wait=[w], on_update=[])
                        newl.append(nop)
                    inst.sync_info = mybir.SyncInfo(on_wait=[waits[-1]],
                                                    on_update=list(si.on_update))
                newl.append(inst)
            if changed:
                bb.instructions = newl


def _build_bass(split_waits=True):
    nc = bass.Bass("TRN2", target_bir_lowering=False, debug=False)
    dt = mybir.dt
    f32 = dt.float32

    a2 = nc.dram_tensor("a2", [N_ATOMS, F * BPAD], f32, kind="ExternalInput").ap()
    idx = nc.dram_tensor("idx", [EPC, 2], dt.int32, kind="ExternalInput").ap()
    disp = nc.dram_tensor("disp", [EPC, 4], f32, kind="ExternalInput").ap()
    wmat = nc.dram_tensor("wmat", [8 * KBLK, 2 * NC_OUT], f32, kind="ExternalInput").ap()
    out = nc.dram_tensor("out", [EPC, 800], f32, kind="ExternalOutput").ap()

    NCHUNK = (ZCOLS + 127) // 128  # 30 zT chunks (last is 32 rows)
    SUPER = 4                      # e-blocks per superblock
    zdt = ZDT

    from contextlib import ExitStack
    with TileContext(nc) as tc, ExitStack() as ctx:
        consts = ctx.enter_context(tc.tile_pool(name="consts", bufs=1))
        wpool = ctx.enter_context(tc.tile_pool(name="wpool", bufs=1))
        epool = ctx.enter_context(tc.tile_pool(name="epool", bufs=3))   # per-eblock working tiles
        spool = ctx.enter_context(tc.tile_pool(name="spool", bufs=3))   # small per-eblock stats
        zpool = ctx.enter_context(tc.tile_pool(name="zpool", bufs=2))  # x4 tags = 8 slots   # Z tiles
        ztp = ctx.enter_context(tc.tile_pool(name="ztp", bufs=2))       # zT sbuf chunks
        opool = ctx.enter_context(tc.tile_pool(name="opool", bufs=3))   # out sbuf
        osp = ctx.enter_context(tc.tile_pool(name="osp", bufs=2))       # [100,512] staging
        pst = ctx.enter_context(tc.tile_pool(name="pst", bufs=3, space="PSUM"))  # transposes of Z
        psm = ctx.enter_context(tc.tile_pool(name="psm", bufs=2, space="PSUM"))  # z-matmul out
        pso = ctx.enter_context(tc.tile_pool(name="pso", bufs=2, space="PSUM"))  # out transposes

        # ---- constants ----
        ident = consts.tile([128, 128], f32)
        make_identity(nc, ident[:])
        identb = consts.tile([128, 128], zdt)
        make_identity(nc, identb[:])
        biasC = consts.tile([128, 1], f32)
        nc.vector.memset(biasC[:], CUTOFF)
        krow = consts.tile([128, F], f32)
        kint = consts.tile([128, F], dt.int32)
        nc.gpsimd.iota(kint[:], pattern=[[1, F]], base=1, channel_multiplier=0)
        nc.vector.tensor_copy(out=krow[:], in_=kint[:])  # 1..16 as float

        # stationary W tiles, one per (f-pair, piece)
        wt = {}
        for fp in range(8):
            for pi, (chunk, r0, r1) in enumerate(_kpieces(fp)):
                t = wpool.tile([r1 - r0, 2 * NC_OUT], zdt, tag=f"w_{fp}_{pi}",
                               name=f"w_{fp}_{pi}")
                base = fp * KBLK + sum(
                    p[2] - p[1] for p in _kpieces(fp)[:pi])
                nc.gpsimd.dma_start(out=t[:], in_=wmat[base:base + (r1 - r0), :])
                wt[(fp, pi)] = t

        for sb in range((NBLK + SUPER - 1) // SUPER):
            eblocks = [eb for eb in range(sb * SUPER, min((sb + 1) * SUPER, NBLK))]
            zs = []
            for eb in eblocks:
                e0 = eb * EBLK
                # ---- gather both endpoints, summed in-DMA ----
                idx_t = spool.tile([128, 2], dt.int32, tag="idx")
                nc.sync.dma_start(out=idx_t[:], in_=idx[e0:e0 + 128, :])
                y = epool.tile([128, F * BPAD], f32, tag="y")
                nc.gpsimd.indirect_dma_start(
                    out=y[:], out_offset=None, in_=a2[:],
                    in_offset=bass.IndirectOffsetOnAxis(ap=idx_t[:, 0:1], axis=0))
                nc.gpsimd.indirect_dma_start(
                    out=y[:], out_offset=None, in_=a2[:],
                    in_offset=bass.IndirectOffsetOnAxis(ap=idx_t[:, 1:2], axis=0),
                    compute_op=mybir.AluOpType.add)

                # ---- per-edge geometry ----
                d = spool.tile([128, 4], f32, tag="d")
                nc.sync.dma_start(out=d[:], in_=disp[e0:e0 + 128, :])
                sq = spool.tile([128, 3], f32, tag="sq")
                nc.scalar.square(sq[:], d[:, 0:3])
                r2 = spool.tile([128, 1], f32, tag="r2")
                nc.vector.tensor_reduce(out=r2[:], in_=sq[:], op=mybir.AluOpType.add,
                                        axis=mybir.AxisListType.X)
                r = spool.tile([128, 1], f32, tag="r")
                nc.scalar.sqrt(r[:], r2[:])
                rm = spool.tile([128, 1], f32, tag="rm")
                nc.vector.tensor_scalar(out=rm[:], in0=r[:], scalar1=1e-9, scalar2=None,
                                        op0=mybir.AluOpType.max)
                rinv = spool.tile([128, 1], f32, tag="rinv")
                nc.vector.reciprocal(rinv[:], rm[:])
                u = spool.tile([128, 3], f32, tag="u")
                nc.vector.tensor_scalar(out=u[:], in0=d[:, 0:3], scalar1=rinv[:, 0:1], scalar2=None,
                                        op0=mybir.AluOpType.mult)
                # mask = (r < CUTOFF) via sign(C - r): {-1,0,1} -> {0,0.5,1}
                msgn = spool.tile([128, 1], f32, tag="msgn")
                nc.scalar.activation(msgn[:], r[:], mybir.ActivationFunctionType.Sign,
                                     bias=biasC[:, 0:1], scale=-1.0)
                mask = spool.tile([128, 1], f32, tag="mask")
                nc.vector.tensor_scalar(out=mask[:], in0=msgn[:], scalar1=0.5,
                                        scalar2=0.5, op0=mybir.AluOpType.mult,
                                        op1=mybir.AluOpType.add)

                # sh [128, 9]
                c1 = 0.4886025119029199
                c2 = 1.0925484305920792
                sh = spool.tile([128, NSH], f32, tag="sh")
                nc.vector.memset(sh[:, 0:1], 0.28209479177387814)
                nc.vector.tensor_scalar(out=sh[:, 1:2], in0=u[:, 1:2], scalar1=c1, scalar2=None,
                                        op0=mybir.AluOpType.mult)
                nc.vector.tensor_scalar(out=sh[:, 2:3], in0=u[:, 2:3], scalar1=c1, scalar2=None,
                                        op0=mybir.AluOpType.mult)
                nc.vector.tensor_scalar(out=sh[:, 3:4], in0=u[:, 0:1], scalar1=c1, scalar2=None,
                                        op0=mybir.AluOpType.mult)
                # xy, yz, xz
                nc.vector.scalar_tensor_tensor(
                    out=sh[:, 4:5], in0=u[:, 0:1], scalar=c2,
                    in1=u[:, 1:2], op0=mybir.AluOpType.mult, op1=mybir.AluOpType.mult)
                nc.vector.scalar_tensor_tensor(
                    out=sh[:, 5:6], in0=u[:, 1:2], scalar=c2,
                    in1=u[:, 2:3], op0=mybir.AluOpType.mult, op1=mybir.AluOpType.mult)
                nc.vector.scalar_tensor_tensor(
                    out=sh[:, 7:8], in0=u[:, 0:1], scalar=c2,
                    in1=u[:, 2:3], op0=mybir.AluOpType.mult, op1=mybir.AluOpType.mult)
                # 0.3154*(3z^2-1)
                t6 = spool.tile([128, 1], f32, tag="t6")
                nc.vector.scalar_tensor_tensor(
                    out=t6[:], in0=u[:, 2:3], scalar=3.0, in1=u[:, 2:3],
                    op0=mybir.AluOpType.mult, op1=mybir.AluOpType.mult)
                nc.scalar.activation(sh[:, 6:7], t6[:], mybir.ActivationFunctionType.Copy,
                                     bias=-0.31539156525252005, scale=0.31539156525252005)
                # 0.5*c2*(x^2-y^2)
                t8 = spool.tile([128, 1], f32, tag="t8")
                nc.vector.scalar_tensor_tensor(
                    out=t8[:], in0=u[:, 0:1], scalar=0.5 * c2, in1=u[:, 0:1],
                    op0=mybir.AluOpType.mult, op1=mybir.AluOpType.mult)
                t8b = spool.tile([128, 1], f32, tag="t8b")
                nc.vector.scalar_tensor_tensor(
                    out=t8b[:], in0=u[:, 1:2], scalar=-0.5 * c2, in1=u[:, 1:2],
                    op0=mybir.AluOpType.mult, op1=mybir.AluOpType.mult)
                nc.vector.tensor_add(out=sh[:, 8:9], in0=t8[:], in1=t8b[:])

                # rad [128, 16]: sinc(k*r/C) * (r<C)
                x = spool.tile([128, F], f32, tag="x")
                rc = spool.tile([128, 1], f32, tag="rc")
                nc.vector.tensor_scalar(out=rc[:], in0=rm[:], scalar1=1.0 / CUTOFF, scalar2=None,
                                        op0=mybir.AluOpType.mult)
                nc.vector.tensor_scalar(out=x[:], in0=krow[:], scalar1=rc[:, 0:1], scalar2=None,
                                        op0=mybir.AluOpType.mult)
                # sin(pi*t) via range reduction: s = t - 2*int(t/2) (trunc or
                # round both keep sin(pi*s) == sin(pi*t) up to period), s in [-1,1]
                px = spool.tile([128, F], f32, tag="px")
                nc.scalar.activation(px[:], x[:], mybir.ActivationFunctionType.Copy,
                                     bias=0.0, scale=math.pi)
                prec = spool.tile([128, F], f32, tag="prec")
                nc.vector.reciprocal(prec[:], px[:])
                # n = round_nearest(x/2) via the 2^23 magic-number trick,
                # s = x - 2n in [-1, 1]; sin(pi*s) == sin(pi*x) by periodicity
                MAGIC = 8388608.0
                th = spool.tile([128, F], f32, tag="th")
                nc.vector.tensor_scalar(out=th[:], in0=x[:], scalar1=0.5,
                                        scalar2=MAGIC, op0=mybir.AluOpType.mult,
                                        op1=mybir.AluOpType.add)
                tf = spool.tile([128, F], f32, tag="tf")
                nc.vector.tensor_scalar(out=tf[:], in0=th[:], scalar1=-MAGIC,
                                        scalar2=None, op0=mybir.AluOpType.add)
                q = spool.tile([128, F], f32, tag="q")
                nc.vector.scalar_tensor_tensor(
                    out=q[:], in0=tf[:], scalar=-2.0, in1=x[:],
                    op0=mybir.AluOpType.mult, op1=mybir.AluOpType.add)
                sins = spool.tile([128, F], f32, tag="sins")
                nc.scalar.activation(sins[:], q[:], mybir.ActivationFunctionType.Sin,
                                     bias=0.0, scale=math.pi)
                rad = spool.tile([128, F], f32, tag="rad")
                nc.vector.scalar_tensor_tensor(
                    out=rad[:], in0=sins[:], scalar=mask[:, 0:1], in1=prec[:],
                    op0=mybir.AluOpType.mult, op1=mybir.AluOpType.mult)

                # ---- y' = y * rad (broadcast over b) ----
                yp = epool.tile([128, F * BPAD], f32, tag="yp")
                nc.gpsimd.tensor_tensor(
                    out=yp[:].rearrange("p (f b) -> p f b", f=F),
                    in0=y[:].rearrange("p (f b) -> p f b", f=F),
                    in1=rad[:, :, None].to_broadcast([128, F, BPAD]),
                    op=mybir.AluOpType.mult)

                # ---- Z[e, (f, a, b)] = sh_a * y' ----
                ebi = eb - eblocks[0]
                z = zpool.tile([128, ZCOLS], zdt, tag=f"z{ebi}", name=f"z{ebi}")
                zs.append(z)
                zap = z[:]
                ypap = yp[:]
                nc.gpsimd.memset(
                    AP(zap.tensor, zap.offset + 2 * ABLK,
                       [list(zap.ap[0]), [FPBLK, 8], [1, FPBLK - 2 * ABLK]]), 0.0)
                for a in range(NSH):
                    zsl = AP(zap.tensor, zap.offset + a * BPAD,
                             [list(zap.ap[0]), [FPBLK, 8], [ABLK, 2], [1, BPAD]])
                    ysl = AP(ypap.tensor, ypap.offset,
                             [list(ypap.ap[0]), [2 * BPAD, 8], [BPAD, 2], [1, BPAD]])
                    if a < 4:
                        nc.vector.tensor_scalar(
                            out=zsl, in0=ysl,
                            scalar1=sh[:, a:a + 1], scalar2=None,
                            op0=mybir.AluOpType.mult)
                    elif a < 6:
                        nc.scalar.activation(
                            zsl, ysl, mybir.ActivationFunctionType.Copy,
                            bias=0.0, scale=sh[:, a:a + 1])
                    else:
                        nc.gpsimd.tensor_scalar(
                            out=zsl, in0=ysl,
                            scalar1=sh[:, a:a + 1], scalar2=None,
                            op0=mybir.AluOpType.mult)

                # ---- transpose Z into zT chunks ----
            nebs = len(eblocks)
            ne = nebs * 128
            # ---- transpose all Z chunks (one wide psum->sbuf copy per chunk) ----
            zts = [ztp.tile([128, 512], zdt, tag=f"zt_{c}", name=f"zt_{c}")
                   for c in range(NCHUNK)]
            for c in range(NCHUNK):
                pt = pst.tile([128, 512], zdt, tag="pt", space="PSUM")
                for ebi in range(nebs):
                    nc.tensor.transpose(out=pt[:, ebi * 128:(ebi + 1) * 128],
                                        in_=zs[ebi][:, c * 128:(c + 1) * 128],
                                        identity=identb[:])
                if c % 4 == 3:
                    nc.scalar.copy(out=zts[c][:, :ne], in_=pt[:, :ne])
                else:
                    nc.vector.tensor_copy(out=zts[c][:, :ne], in_=pt[:, :ne])
            # ---- z-matmul per f-pair + transpose back ----
            outs = [opool.tile([128, 800], f32, tag=f"os_{i}", name=f"os_{i}")
                    for i in range(nebs)]
            for fp in range(8):
                po = psm.tile([NC_OUT * 2, 512], f32, tag="po", space="PSUM")
                pieces = _kpieces(fp)
                for pi, (chunk, r0, r1) in enumerate(pieces):
                    nc.tensor.matmul(
                        out=po[:, :ne], lhsT=wt[(fp, pi)][:],
                        rhs=zts[chunk][r0:r1, :ne],
                        start=(pi == 0), stop=(pi == len(pieces) - 1))
                og = osp.tile([NC_OUT * 2, 512], f32, tag="og")
                nc.scalar.copy(out=og[:, :ne], in_=po[:, :ne])
                for ebi in range(nebs):
                    pt2 = pso.tile([128, NC_OUT * 2], f32, tag="pt2", space="PSUM")
                    nc.tensor.transpose(out=pt2[:, :],
                                        in_=og[:, ebi * 128:(ebi + 1) * 128],
                                        identity=ident[:NC_OUT * 2, :NC_OUT * 2])
                    # scatter into out sbuf: col = c*16 + 2*fp + df
                    orr = outs[ebi][:].rearrange("p (c k) -> p c k", k=16)
                    if (fp + ebi) % 4 == 3:
                        nc.scalar.activation(
                            orr[:, :, 2 * fp:2 * fp + 2],
                            pt2[:].rearrange("p (c t) -> p c t", t=2),
                            mybir.ActivationFunctionType.Copy, bias=0.0, scale=1.0)
                    else:
                        nc.vector.tensor_copy(
                            out=orr[:, :, 2 * fp:2 * fp + 2],
                            in_=pt2[:].rearrange("p (c t) -> p c t", t=2))
            for ebi, eb in enumerate(eblocks):
                e0 = eb * EBLK
                nc.sync.dma_start(out=out[e0:e0 + 128, :], in_=outs[ebi][:])

    if split_waits:
        _split_multi_waits(nc)
    return nc


def _get_nc():
    if "nc" not in _NC_CACHE:
        _NC_CACHE["nc"] = _build_bass()
    return _NC_CACHE["nc"]


# ----------------------------------------------------------------------------
# Host entry point
# ----------------------------------------------------------------------------
def kernel(atomic_descriptors, tp_weights, neighbour_displacements,
           neighbour_indices):
    atomic_descriptors = np.asarray(atomic_descriptors, dtype=np.float32)
    tp_weights = np.asarray(tp_weights, dtype=np.float32)
    neighbour_displacements = np.asarray(neighbour_displacements, dtype=np.float32)
    neighbour_indices = np.asarray(neighbour_indices, dtype=np.int32)

    # relayout atom table: (A, 1, 25, 16) -> (A, 16, 26) f-major, b padded
    A = atomic_descriptors.reshape(N_ATOMS, NB, F)
    a2 = np.zeros((N_ATOMS, F, BPAD), dtype=np.float32)
    a2[:, :, :NB] = A.transpose(0, 2, 1)
    a2 = a2.reshape(N_ATOMS, F * BPAD)

    wmat = _build_weight_tensor(tp_weights).astype(np.float32)

    in_maps = []
    shard = N_EDGES // N_CORES
    for c in range(N_CORES):
        idx = np.zeros((EPC, 2), dtype=np.int32)
        disp = np.zeros((EPC, 4), dtype=np.float32)
        idx[:shard] = neighbour_indices[c * shard:(c + 1) * shard]
        d = neighbour_displacements[c * shard:(c + 1) * shard]
        disp[:shard, :3] = d
        disp[shard:, :3] = 1.0  # harmless dummy
        in_maps.append({"a2": a2, "idx": idx, "disp": disp, "wmat": wmat})

    global _last_in_maps
    _last_in_maps = in_maps
    nc = _get_nc()
    res = run_bass_kernel_spmd(nc, in_maps, core_ids=list(range(N_CORES)))

    out = np.empty((N_EDGES, 2, NB, F), dtype=np.float32)
    for c in range(N_CORES):
        o = res.results[c]["out"][:shard].reshape(shard, 2, NB, F)
        out[c * shard:(c + 1) * shard] = o
    return out


if __name__ == "__main__":
    rng = np.random.default_rng(0)
    inputs = {
        "atomic_descriptors": rng.standard_normal((N_ATOMS, 1, NB, F), dtype=np.float32),
        "tp_weights": (rng.standard_normal((len(PATHS), F)) * 0.1).astype(np.float32),
        "neighbour_displacements": (rng.standard_normal((N_EDGES, 3)) * 1.5).astype(np.float32),
        "neighbour_indices": rng.integers(0, N_ATOMS, (N_EDGES, 2)).astype(np.int32),
    }
    out = kernel(**inputs)
    print("kernel ran, out shape", out.shape)



# revision 32
# speedup vs baseline: 1.5742x; 1.5742x over previous
"""Bond-centered tensor-moment descriptor kernel for Trainium2 (8 NeuronCores).

Strategy: edges are sharded 8 ways; every core gets the full (relaid-out)
atom-descriptor table and gathers its edge endpoints with indirect DMA
(one indirect DMA per endpoint per 4-block superblock, summed in-DMA).
The Clebsch-Gordan tensor product is computed as: build Z[e,(fp,df,a,b)] =
sh_a(u)*rad_f(r)*y_b,f with per-partition-scalar ops, transpose Z to
feature-partitions with TensorE, then one accumulating matmul chain per
f-pair whose stationary weights fold CG coefficients and tp_weights.
The [100, e] matmul results are DMAed to DRAM untransposed; the host
does the final (free) relayout to [E, 2, 25, 16].
"""
import math
import numpy as np

import concourse.bass as bass
import concourse.tile as tile
from concourse import mybir
from concourse.bass import AP
from concourse.bass_utils import run_bass_kernel_spmd
from concourse.masks import make_identity
from concourse.tile import TileContext, ScopedClock

# ----------------------------------------------------------------------------
# Problem constants (hardcoded per contract)
# ----------------------------------------------------------------------------
CUTOFF = 5.0
MAX_BASIS_DEG = 2
MAX_DEG = 4
N_ATOMS = 20000
N_EDGES = 50000
F = 16
N_CORES = 8

NSH = (MAX_BASIS_DEG + 1) ** 2        # 9 spherical-harmonic components
NB = (MAX_DEG + 1) ** 2               # 25 atom-feature m-slots
BPAD = 26                             # b padded for alignment
NC_OUT = 2 * NB                       # 50 output (parity, c) slots
ABLK = NSH * BPAD                     # 234 Z-columns per f
KUSE = 2 * ABLK                       # 468 useful Z-columns per f-pair
FPBLK = 512                           # padded Z-columns per f-pair
ZCOLS = 8 * FPBLK                     # 4096
EPC = 6400                            # edges per core (padded from 6250)
EBLK = 128                            # edges per block
NBLK = EPC // EBLK                    # 50 blocks per core
SUPER = 4                             # e-blocks per superblock
NCHUNK = ZCOLS // 128                 # 32 zT chunks (4 per f-pair)
GROW = F * BPAD                       # gather row (bf16)
NSB_C = (NBLK + SUPER - 1) // SUPER   # superblocks per core

PATHS = [(l1, l2, l3)
         for l1 in range(MAX_BASIS_DEG + 1)
         for l2 in range(MAX_DEG + 1)
         for l3 in range(abs(l1 - l2), min(l1 + l2, MAX_DEG) + 1)]

ZDT = mybir.dt.bfloat16               # Z / zT / W dtype


# ----------------------------------------------------------------------------
# Clebsch-Gordan coefficients (host, numpy only)
# ----------------------------------------------------------------------------
def _fac(n):
    return math.factorial(n)


def _cg(j1, m1, j2, m2, j3, m3):
    if m1 + m2 != m3:
        return 0.0
    if j3 < abs(j1 - j2) or j3 > j1 + j2:
        return 0.0
    pre = math.sqrt((2 * j3 + 1) * _fac(j3 + j1 - j2) * _fac(j3 - j1 + j2)
                    * _fac(j1 + j2 - j3) / _fac(j1 + j2 + j3 + 1))
    pre *= math.sqrt(_fac(j3 + m3) * _fac(j3 - m3) * _fac(j1 - m1) * _fac(j1 + m1)
                     * _fac(j2 - m2) * _fac(j2 + m2))
    s = 0.0
    for k in range(max(0, j2 - j3 - m1, j1 - j3 + m2),
                   min(j1 + j2 - j3, j1 - m1, j2 + m2) + 1):
        s += (-1) ** k / (_fac(k) * _fac(j1 + j2 - j3 - k) * _fac(j1 - m1 - k)
                          * _fac(j2 + m2 - k) * _fac(j3 - j2 + m1 + k)
                          * _fac(j3 - j1 - m2 + k))
    return pre * s


def _umat(l):
    U = np.zeros((2 * l + 1, 2 * l + 1), dtype=np.complex128)
    s2 = 1.0 / np.sqrt(2.0)
    for m in range(-l, l + 1):
        if m > 0:
            U[m + l, m + l] = ((-1) ** m) * s2
            U[m + l, -m + l] = s2
        elif m == 0:
            U[l, l] = 1.0
        else:
            am = -m
            U[m + l, m + l] = 1j * s2
            U[m + l, am + l] = -1j * ((-1) ** am) * s2
    return U


def _real_cg(l1, l2, l3):
    C = np.zeros((2 * l1 + 1, 2 * l2 + 1, 2 * l3 + 1), dtype=np.complex128)
    for m1 in range(-l1, l1 + 1):
        for m2 in range(-l2, l2 + 1):
            m3 = m1 + m2
            if -l3 <= m3 <= l3:
                C[m1 + l1, m2 + l2, m3 + l3] = _cg(l1, m1, l2, m2, l3, m3)
    G = np.einsum('aA,bB,cC,ABC->abc', _umat(l1), _umat(l2),
                  np.conj(_umat(l3)), C)
    G = G.real if (l1 + l2 + l3) % 2 == 0 else G.imag
    return np.ascontiguousarray(G)


def _build_weight_tensor(tp_weights):
    """W -> [8*FPBLK, 2*NC_OUT] in Z-column order (fp, df, a, b-padded),
    entry = CG[a,b,c] * tp_weights[path, f], f = 2*fp+df, output column
    q = 2*c + df."""
    G_abc = np.zeros((NSH, NB, NC_OUT), dtype=np.float64)
    for p, (l1, l2, l3) in enumerate(PATHS):
        G = _real_cg(l1, l2, l3)
        par = (l1 + l2 + l3) % 2
        for ai in range(2 * l1 + 1):
            for bi in range(2 * l2 + 1):
                for ci in range(2 * l3 + 1):
                    v = G[ai, bi, ci]
                    if v != 0.0:
                        ga = l1 * l1 + ai
                        gb = l2 * l2 + bi
                        gc = par * NB + l3 * l3 + ci
                        G_abc[ga, gb, gc] = v
    path_idx = {}
    for p, (l1, l2, l3) in enumerate(PATHS):
        path_idx[(l1, l2, l3)] = p
    l_of_a = [0, 1, 1, 1, 2, 2, 2, 2, 2]
    l_of_b = [int(np.sqrt(b)) for b in range(NB)]
    l_of_c = [int(np.sqrt(c % NB)) for c in range(NC_OUT)]

    W = np.zeros((F, NSH, BPAD, NC_OUT), dtype=np.float64)
    for ga in range(NSH):
        for gb in range(NB):
            nz = np.nonzero(G_abc[ga, gb])[0]
            if len(nz) == 0:
                continue
            for gc in nz:
                p = path_idx[(l_of_a[ga], l_of_b[gb], l_of_c[gc])]
                for f in range(F):
                    W[f, ga, gb, gc] = G_abc[ga, gb, gc] * float(tp_weights[p, f])
    W = W.reshape(F, ABLK, NC_OUT)
    out = np.zeros((8, FPBLK, 2 * NC_OUT), dtype=np.float64)
    for fp in range(8):
        for df in range(2):
            out[fp, df * ABLK:(df + 1) * ABLK, df::2] = W[2 * fp + df]
    return out.reshape(8 * FPBLK, 2 * NC_OUT)


# ----------------------------------------------------------------------------
# Device kernel builder
# ----------------------------------------------------------------------------
_NC_CACHE = {}


def _drain_and_barrier_patched(self, tick_clock, wait_clock):
    # this container's walrus supports only one sync-wait per CTRL
    nc = self.nc
    drain_inst = nc.sync.drain()
    wait_clock.add_sem_waits(drain_inst.ins,
                             ScopedClock({None: tick_clock.global_clock}))
    si = drain_inst.ins.sync_info
    waits = list(si.on_wait) if si else []
    if len(waits) > 1:
        drain_inst.ins.sync_info = mybir.SyncInfo(on_wait=[waits[0]],
                                                  on_update=list(si.on_update))
        for w in waits[1:]:
            d2 = nc.sync.drain()
            d2.ins.sync_info = mybir.SyncInfo(on_wait=[w], on_update=[])
    nc.all_engine_barrier()
    assert self.sems is not None
    popped = nc._tile_sem_poison_stack.pop()
    assert popped is self._sem_poison
    nc.clear_and_free_semaphores(list(self.sems.allocated().values()))
    nc.all_engine_barrier()


TileContext._drain_and_barrier = _drain_and_barrier_patched

# per-f-pair contraction pieces: (chunk, rows); last piece only 84 useful rows
_PIECES = [(0, 128), (1, 128), (2, 128), (3, KUSE - 3 * 128)]

# engine assignment knobs ('v' = DVE, 'a' = Act, 'p' = Pool/gpsimd)
# NOTE: gpsimd cannot read PSUM, so BPAIR/OG must use only 'v'/'a'
ZSLICE_ENG = ['v', 'v', 'v', 'v', 'v', 'a', 'a', 'p', 'p']
BPAIR_ENG = ['v', 'a', 'v', 'a', 'v', 'a', 'v', 'v',
             'v', 'a', 'v', 'a', 'v', 'a', 'v', 'v']
OG_ENG = ['a', 'v', 'a', 'a']


def _split_multi_waits(nc):
    """This container's walrus supports one sync-wait per instruction; move
    extra waits onto injected same-engine NoOps placed just before."""
    for f in nc.m.functions:
        for bb in f.blocks:
            newl = []
            changed = False
            for inst in bb.instructions:
                si = inst.sync_info
                waits = list(si.on_wait) if si else []
                if len(waits) > 1:
                    changed = True
                    for k, w in enumerate(waits[:-1]):
                        nop = mybir.InstDrain(name=f"{inst.name}-sw{k}",
                                              ins=[], outs=[])
                        nop.engine = inst.engine
                        nop.sync_info = mybir.SyncInfo(on_wait=[w], on_update=[])
                        newl.append(nop)
                    inst.sync_info = mybir.SyncInfo(on_wait=[waits[-1]],
                                                    on_update=list(si.on_update))
                newl.append(inst)
            if changed:
                bb.instructions = newl


def _build_bass(split_waits=True):
    nc = bass.Bass("TRN2", target_bir_lowering=False, debug=False)
    dt = mybir.dt
    f32 = dt.float32
    i32 = dt.int32
    zdt = ZDT
    AF = mybir.ActivationFunctionType
    OP = mybir.AluOpType

    a2 = nc.dram_tensor("a2", [N_ATOMS, GROW], ZDT, kind="ExternalInput").ap()
    idx = nc.dram_tensor("idx", [EPC * 2], i32, kind="ExternalInput").ap()
    disp = nc.dram_tensor("disp", [EPC * 4], f32, kind="ExternalInput").ap()
    wmat = nc.dram_tensor("wmat", [8 * FPBLK, 2 * NC_OUT], ZDT,
                          kind="ExternalInput").ap()
    out2 = nc.dram_tensor("out2", [2 * NC_OUT * 8 * EPC], f32,
                          kind="ExternalOutput").ap()

    NSB = NSB_C
    c1 = 0.4886025119029199
    c2 = 1.0925484305920792
    MAGICI = 0x5f3759df
    MAGICF = 8388608.0

    from contextlib import ExitStack
    with TileContext(nc) as tc, ExitStack() as ctx:
        consts = ctx.enter_context(tc.tile_pool(name="consts", bufs=1))
        wpool = ctx.enter_context(tc.tile_pool(name="wpool", bufs=1))
        epool = ctx.enter_context(tc.tile_pool(name="epool", bufs=2))   # y gather
        ypp = ctx.enter_context(tc.tile_pool(name="ypp", bufs=3))       # yp tiles
        spool = ctx.enter_context(tc.tile_pool(name="spool", bufs=2))   # geometry
        zpool = ctx.enter_context(tc.tile_pool(name="zpool", bufs=2))   # Z tiles
        ztp = ctx.enter_context(tc.tile_pool(name="ztp", bufs=1))       # zT sbuf
        opool = ctx.enter_context(tc.tile_pool(name="opool", bufs=2))   # og sbuf
        pst = ctx.enter_context(tc.tile_pool(name="pst", bufs=4, space="PSUM"))
        psm = ctx.enter_context(tc.tile_pool(name="psm", bufs=2, space="PSUM"))

        # ---- constants ----
        identb = consts.tile([128, 128], zdt)
        make_identity(nc, identb[:])
        biasC2 = consts.tile([128, 1], f32)
        nc.vector.memset(biasC2[:], CUTOFF * CUTOFF)
        # krc4[p, ebi, f] = (f+1)/CUTOFF for ebi in 0..3
        kint = consts.tile([128, F], i32)
        nc.gpsimd.iota(kint[:], pattern=[[1, F]], base=1, channel_multiplier=0)
        krc4 = consts.tile([128, SUPER * F], f32)
        for s_ in range(SUPER):
            nc.vector.tensor_copy(out=krc4[:, s_ * F:(s_ + 1) * F],
                                  in_=kint[:])
        nc.vector.tensor_scalar(out=krc4[:], in0=krc4[:], scalar1=1.0 / CUTOFF,
                                scalar2=None, op0=OP.mult)

        # stationary W: one [128, 32*100] tile, one DMA. Block (4*fp+pi)
        # holds piece rows [fp*512+pi*128, +128) (reading into the zero pad
        # past row 468 is harmless; matmul reads only the useful rows).
        wt_all = consts.tile([128, 32 * 2 * NC_OUT], zdt)
        nc.sync.dma_start(
            out=wt_all[:],
            in_=AP(wmat.tensor, 0,
                   [[2 * NC_OUT, 128], [128 * 2 * NC_OUT, 32], [1, 2 * NC_OUT]]))

        def wt(fp, pi):
            blk = 4 * fp + pi
            rows = _PIECES[pi][1]
            return wt_all[:rows, blk * 2 * NC_OUT:(blk + 1) * 2 * NC_OUT]

        for sb in range(NSB):
            eblocks = list(range(sb * SUPER, min((sb + 1) * SUPER, NBLK)))
            nebs = len(eblocks)
            ne = nebs * 128
            e0 = sb * SUPER * EBLK

            # ---- load idx/disp for the superblock ----
            idx_t = spool.tile([128, SUPER, 2], i32, tag="idxt", name="idxt")
            nc.sync.dma_start(
                out=idx_t[:, :nebs, :],
                in_=AP(idx.tensor, e0 * 2,
                       [[2, 128], [256, nebs], [1, 2]]))
            d4 = spool.tile([128, SUPER, 4], f32, tag="d4")
            nc.sync.dma_start(
                out=d4[:, :nebs, :],
                in_=AP(disp.tensor, e0 * 4,
                       [[4, 128], [512, nebs], [1, 4]]))

            # ---- per-eblock endpoint gathers (single-offset-per-partition
            # indirect DMA, baseline-proven), second endpoint added in-DMA ----
            y2 = epool.tile([128, SUPER * GROW], zdt, tag="y2")
            y2ap = y2[:]
            yv = y2ap.rearrange("p (s w) -> p s w", s=SUPER)
            for ebi in range(nebs):
                nc.gpsimd.indirect_dma_start(
                    out=yv[:, ebi, :], out_offset=None, in_=a2[:],
                    in_offset=bass.IndirectOffsetOnAxis(
                        ap=idx_t[:, ebi, 0:1], axis=0))
                nc.gpsimd.indirect_dma_start(
                    out=yv[:, ebi, :], out_offset=None, in_=a2[:],
                    in_offset=bass.IndirectOffsetOnAxis(
                        ap=idx_t[:, ebi, 1:2], axis=0),
                    compute_op=OP.add)

            # ---- geometry for the whole superblock ([128, nebs*k] tiles) ----
            def st(k, name, dtype=f32):
                return spool.tile([128, SUPER, k], dtype, tag=name, name=name)

            sq = st(3, "sq")
            nc.scalar.square(sq[:, :nebs, :], d4[:, :nebs, 0:3])
            r2 = st(1, "r2")
            nc.vector.tensor_reduce(out=r2[:, :nebs, :], in_=sq[:, :nebs, :],
                                    op=OP.add, axis=mybir.AxisListType.X)
            # rsqrt via bit trick + 2 Newton steps (keeps sqrt off ActE)
            yi = st(1, "yi", i32)
            nc.vector.tensor_scalar(out=yi[:, :nebs, :],
                                    in0=r2[:, :nebs, :].bitcast(i32),
                                    scalar1=1, scalar2=None,
                                    op0=OP.logical_shift_right)
            nc.vector.tensor_scalar(out=yi[:, :nebs, :], in0=yi[:, :nebs, :],
                                    scalar1=-1, scalar2=None,
                                    op0=OP.bitwise_xor)
            nc.vector.tensor_scalar(out=yi[:, :nebs, :], in0=yi[:, :nebs, :],
                                    scalar1=MAGICI + 1, scalar2=None,
                                    op0=OP.add)
            rv = st(1, "rv")
            nc.vector.tensor_copy(out=rv[:, :nebs, :],
                                  in_=yi[:, :nebs, :].bitcast(f32))
            nt = st(1, "nt")
            for _ in range(2):
                nc.vector.tensor_tensor(out=nt[:, :nebs, :], in0=r2[:, :nebs, :],
                                        in1=rv[:, :nebs, :], op=OP.mult)
                nc.vector.tensor_tensor(out=nt[:, :nebs, :], in0=nt[:, :nebs, :],
                                        in1=rv[:, :nebs, :], op=OP.mult)
                nc.vector.tensor_scalar(out=nt[:, :nebs, :], in0=nt[:, :nebs, :],
                                        scalar1=-0.5, scalar2=1.5,
                                        op0=OP.mult, op1=OP.add)
                nc.vector.tensor_tensor(out=rv[:, :nebs, :], in0=rv[:, :nebs, :],
                                        in1=nt[:, :nebs, :], op=OP.mult)
            r = st(1, "r")
            nc.vector.tensor_tensor(out=r[:, :nebs, :], in0=r2[:, :nebs, :],
                                    in1=rv[:, :nebs, :], op=OP.mult)
            # mask = (r2 < C^2) via sign
            msgn = st(1, "msgn")
            nc.scalar.activation(msgn[:, :nebs, :], r2[:, :nebs, :], AF.Sign,
                                 bias=biasC2[:, 0:1], scale=-1.0)
            mask = st(1, "mask")
            nc.vector.tensor_scalar(out=mask[:, :nebs, :], in0=msgn[:, :nebs, :],
                                    scalar1=0.5, scalar2=0.5,
                                    op0=OP.mult, op1=OP.add)
            # unit displacement, component-major [128, 3, SUPER] so Pool ops
            # downstream see unit-stride last dims (Pool codegen requirement)
            u3 = spool.tile([128, 3, SUPER], f32, tag="u3", name="u3")
            d4ap = d4[:]
            rvap = rv[:]
            nc.vector.tensor_tensor(
                out=u3[:, :, :nebs],
                in0=AP(d4ap.tensor, d4ap.offset,
                       [list(d4ap.ap[0]), [1, 3], [4, nebs]]),
                in1=AP(rvap.tensor, rvap.offset,
                       [list(rvap.ap[0]), [0, 3], [1, nebs]]),
                op=OP.mult)
            ux, uy, uz = (u3[:, j, :nebs] for j in range(3))

            # spherical harmonics sh[p, a, ebi] (a-major for contiguity)
            sh = spool.tile([128, NSH, SUPER], f32, tag="sh", name="sh")
            nc.vector.memset(sh[:, 0, :nebs], 0.28209479177387814)
            nc.vector.tensor_scalar(out=sh[:, 1, :nebs], in0=uy, scalar1=c1,
                                    scalar2=None, op0=OP.mult)
            nc.vector.tensor_scalar(out=sh[:, 2, :nebs], in0=uz, scalar1=c1,
                                    scalar2=None, op0=OP.mult)
            nc.vector.tensor_scalar(out=sh[:, 3, :nebs], in0=ux, scalar1=c1,
                                    scalar2=None, op0=OP.mult)
            nc.vector.scalar_tensor_tensor(out=sh[:, 4, :nebs], in0=ux,
                                           scalar=c2, in1=uy,
                                           op0=OP.mult, op1=OP.mult)
            nc.vector.scalar_tensor_tensor(out=sh[:, 5, :nebs], in0=uy,
                                           scalar=c2, in1=uz,
                                           op0=OP.mult, op1=OP.mult)
            t6 = st(1, "t6")
            nc.vector.tensor_tensor(out=t6[:, :nebs, 0], in0=uz,
                                    in1=uz, op=OP.mult)
            nc.vector.tensor_scalar(out=sh[:, 6, :nebs], in0=t6[:, :nebs, 0],
                                    scalar1=3.0 * 0.31539156525252005,
                                    scalar2=-0.31539156525252005,
                                    op0=OP.mult, op1=OP.add)
            nc.vector.scalar_tensor_tensor(out=sh[:, 7, :nebs], in0=ux,
                                           scalar=c2, in1=uz,
                                           op0=OP.mult, op1=OP.mult)
            t8 = st(1, "t8")
            nc.vector.scalar_tensor_tensor(out=t8[:, :nebs, 0], in0=ux,
                                           scalar=0.5 * c2, in1=ux,
                                           op0=OP.mult, op1=OP.mult)
            t8b = st(1, "t8b")
            nc.vector.scalar_tensor_tensor(out=t8b[:, :nebs, 0], in0=uy,
                                           scalar=-0.5 * c2, in1=uy,
                                           op0=OP.mult, op1=OP.mult)
            nc.vector.tensor_tensor(out=sh[:, 8, :nebs], in0=t8[:, :nebs, 0],
                                    in1=t8b[:, :nebs, 0], op=OP.add)

            # radial basis rad[p, ebi, f] = sinc(k*r/C) * (r<C)
            x = st(F, "x")
            nc.vector.tensor_tensor(
                out=x[:, :nebs, :],
                in0=krc4[:].rearrange("p (s f) -> p s f", s=SUPER)[:, :nebs, :],
                in1=r[:, :nebs, :].to_broadcast([128, nebs, F]), op=OP.mult)
            px = st(F, "px")
            nc.vector.tensor_scalar(out=px[:, :nebs, :], in0=x[:, :nebs, :],
                                    scalar1=math.pi, scalar2=None, op0=OP.mult)
            prec = st(F, "prec")
            nc.vector.reciprocal(prec[:, :nebs, :], px[:, :nebs, :])
            th = st(F, "th")
            nc.vector.tensor_scalar(out=th[:, :nebs, :], in0=x[:, :nebs, :],
                                    scalar1=0.5, scalar2=MAGICF,
                                    op0=OP.mult, op1=OP.add)
            nc.vector.tensor_scalar(out=th[:, :nebs, :], in0=th[:, :nebs, :],
                                    scalar1=-MAGICF, scalar2=None, op0=OP.add)
            q = st(F, "q")
            nc.vector.scalar_tensor_tensor(out=q[:, :nebs, :],
                                           in0=th[:, :nebs, :], scalar=-2.0,
                                           in1=x[:, :nebs, :],
                                           op0=OP.mult, op1=OP.add)
            sins = st(F, "sins")
            nc.scalar.activation(sins[:, :nebs, :], q[:, :nebs, :], AF.Sin,
                                 bias=0.0, scale=math.pi)
            rad = st(F, "rad")
            nc.vector.tensor_tensor(out=rad[:, :nebs, :], in0=sins[:, :nebs, :],
                                    in1=prec[:, :nebs, :], op=OP.mult)
            nc.gpsimd.tensor_tensor(
                out=rad[:, :nebs, :], in0=rad[:, :nebs, :],
                in1=mask[:, :nebs, :].to_broadcast([128, nebs, F]), op=OP.mult)

            # ---- yp = y*rad for the whole superblock (one DVE instr) ----
            yp = ypp.tile([128, SUPER * F * BPAD], zdt, tag="yp")
            nc.vector.tensor_tensor(
                out=yp[:].rearrange("p (s f b) -> p s f b", s=SUPER, f=F)[
                    :, :nebs, :, :],
                in0=AP(y2ap.tensor, y2ap.offset,
                       [list(y2ap.ap[0]), [GROW, nebs], [BPAD, F], [1, BPAD]]),
                in1=rad[:, :nebs, :, None].to_broadcast([128, nebs, F, BPAD]),
                op=OP.mult)

            # ---- Z slices ----
            zs = []
            for ebi in range(nebs):
                z = zpool.tile([128, ZCOLS], zdt, tag=f"z{ebi}", name=f"z{ebi}")
                zs.append(z)
                zap = z[:]
                ypap = yp[:]
                for a in range(NSH):
                    zsl = AP(zap.tensor, zap.offset + a * BPAD,
                             [list(zap.ap[0]), [FPBLK, 8], [ABLK, 2], [1, BPAD]])
                    ysl = AP(ypap.tensor, ypap.offset + ebi * F * BPAD,
                             [list(ypap.ap[0]), [2 * BPAD, 8], [BPAD, 2],
                              [1, BPAD]])
                    sca = sh[:, a, ebi:ebi + 1]
                    eng = ZSLICE_ENG[a]
                    if eng == 'v':
                        nc.vector.tensor_scalar(out=zsl, in0=ysl, scalar1=sca,
                                                scalar2=None, op0=OP.mult)
                    elif eng == 'a':
                        nc.scalar.activation(zsl, ysl, AF.Copy,
                                             bias=0.0, scale=sca)
                    else:
                        nc.gpsimd.tensor_scalar(out=zsl, in0=ysl, scalar1=sca,
                                                scalar2=None, op0=OP.mult)

            # ---- per fp-pair-group: transpose its 4 chunk-pairs, then its
            # matmuls; interleaving keeps PE busy while copies drain ----
            og = opool.tile([2 * NC_OUT, 8 * 512], f32, tag="og")
            zts = {}
            for fpp in range(4):
                for pr in range(4 * fpp, 4 * fpp + 4):
                    zt = ztp.tile([128, 1024], zdt, tag=f"zt_{pr}",
                                  name=f"zt_{pr}")
                    zts[pr] = zt
                    pt = pst.tile([128, 1024], zdt, tag="pt", space="PSUM")
                    for half in range(2):
                        c = 2 * pr + half
                        rows = _PIECES[c % 4][1]
                        for ebi in range(nebs):
                            nc.tensor.transpose(
                                out=pt[:rows,
                                       half * 512 + ebi * 128:
                                       half * 512 + (ebi + 1) * 128],
                                in_=zs[ebi][:, c * 128:c * 128 + rows],
                                identity=identb[:])
                    eng = BPAIR_ENG[pr]
                    if eng == 'v':
                        nc.vector.tensor_copy(out=zt[:], in_=pt[:])
                    elif eng == 'a':
                        nc.scalar.copy(out=zt[:], in_=pt[:])
                    else:
                        nc.gpsimd.tensor_copy(out=zt[:], in_=pt[:])

                po = psm.tile([2 * NC_OUT, 1024], f32, tag="po", space="PSUM")
                for half in range(2):
                    fp = 2 * fpp + half
                    for pi, (ch, rows) in enumerate(_PIECES):
                        c = 4 * fp + ch
                        nc.tensor.matmul(
                            out=po[:, half * 512:half * 512 + ne],
                            lhsT=wt(fp, pi),
                            rhs=zts[c // 2][:rows,
                                            (c % 2) * 512:(c % 2) * 512 + ne],
                            start=(pi == 0), stop=(pi == len(_PIECES) - 1))
                eng = OG_ENG[fpp]
                if eng == 'v':
                    nc.vector.tensor_copy(out=og[:, fpp * 1024:(fpp + 1) * 1024],
                                          in_=po[:])
                else:
                    nc.scalar.copy(out=og[:, fpp * 1024:(fpp + 1) * 1024],
                                   in_=po[:])
            ogap = og[:]
            nc.sync.dma_start(
                out=AP(out2.tensor, sb * SUPER * 128,
                       [[EPC, 2 * NC_OUT], [2 * NC_OUT * EPC, 8], [1, ne]]),
                in_=AP(ogap.tensor, ogap.offset,
                       [list(ogap.ap[0]), [512, 8], [1, ne]]))

    if split_waits:
        _split_multi_waits(nc)
    return nc


def _get_nc():
    if "nc" not in _NC_CACHE:
        _NC_CACHE["nc"] = _build_bass()
    return _NC_CACHE["nc"]


# ----------------------------------------------------------------------------
# Host entry point
# ----------------------------------------------------------------------------
def kernel(atomic_descriptors, tp_weights, neighbour_displacements,
           neighbour_indices):
    atomic_descriptors = np.asarray(atomic_descriptors, dtype=np.float32)
    tp_weights = np.asarray(tp_weights, dtype=np.float32)
    neighbour_displacements = np.asarray(neighbour_displacements, dtype=np.float32)
    neighbour_indices = np.asarray(neighbour_indices, dtype=np.int32)

    # relayout atom table: (A, 1, 25, 16) -> (A, 16, 26) f-major, b padded
    A = atomic_descriptors.reshape(N_ATOMS, NB, F)
    import ml_dtypes
    a2 = np.zeros((N_ATOMS, F, BPAD), dtype=np.float32)
    a2[:, :, :NB] = A.transpose(0, 2, 1)
    a2 = a2.reshape(N_ATOMS, F * BPAD).astype(ml_dtypes.bfloat16)

    import ml_dtypes
    wmat = _build_weight_tensor(tp_weights).astype(ml_dtypes.bfloat16)

    in_maps = []
    shard = N_EDGES // N_CORES
    for c in range(N_CORES):
        idxp = np.zeros((EPC, 2), dtype=np.int32)
        dispp = np.zeros((EPC, 4), dtype=np.float32)
        idxp[:shard] = neighbour_indices[c * shard:(c + 1) * shard]
        d = neighbour_displacements[c * shard:(c + 1) * shard]
        dispp[:shard, :3] = d
        dispp[shard:, :3] = 1.0  # harmless dummy
        in_maps.append({"a2": a2, "idx": idxp.reshape(-1),
                        "disp": dispp.reshape(-1), "wmat": wmat})

    global _last_in_maps
    _last_in_maps = in_maps
    nc = _get_nc()
    res = run_bass_kernel_spmd(nc, in_maps, core_ids=list(range(N_CORES)))

    out = np.empty((N_EDGES, 2, NB, F), dtype=np.float32)
    for c in range(N_CORES):
        o2 = res.results[c]["out2"].reshape(2 * NC_OUT * 8, EPC)
        # row = fp*100 + 2*(par*25+m) + df ; f = 2*fp + df
        o = o2.reshape(8, NC_OUT, 2, EPC)[:, :, :, :shard]
        o = o.reshape(8, 2, NB, 2, shard)          # (fp, par, m, df, e)
        o = o.transpose(4, 1, 2, 0, 3)             # (e, par, m, fp, df)
        out[c * shard:(c + 1) * shard] = o.reshape(shard, 2, NB, F)
    return out


if __name__ == "__main__":
    rng = np.random.default_rng(0)
    inputs = {
        "atomic_descriptors": rng.standard_normal((N_ATOMS, 1, NB, F), dtype=np.float32),
        "tp_weights": (rng.standard_normal((len(PATHS), F)) * 0.1).astype(np.float32),
        "neighbour_displacements": (rng.standard_normal((N_EDGES, 3)) * 1.5).astype(np.float32),
        "neighbour_indices": rng.integers(0, N_ATOMS, (N_EDGES, 2)).astype(np.int32),
    }
    out = kernel(**inputs)
    print("kernel ran, out shape", out.shape)


# revision 33
# speedup vs baseline: 1.7072x; 1.0845x over previous
"""Bond-centered tensor-moment descriptor kernel for Trainium2 (8 NeuronCores).

Strategy: edges are sharded 8 ways; every core gets the full (relaid-out)
atom-descriptor table and gathers its edge endpoints with indirect DMA
(one indirect DMA per endpoint per 4-block superblock, summed in-DMA).
The Clebsch-Gordan tensor product is computed as: build Z[e,(fp,df,a,b)] =
sh_a(u)*rad_f(r)*y_b,f with per-partition-scalar ops, transpose Z to
feature-partitions with TensorE, then one accumulating matmul chain per
f-pair whose stationary weights fold CG coefficients and tp_weights.
The [100, e] matmul results are DMAed to DRAM untransposed; the host
does the final (free) relayout to [E, 2, 25, 16].
"""
import math
import numpy as np

import concourse.bass as bass
import concourse.tile as tile
from concourse import mybir
from concourse.bass import AP
from concourse.bass_utils import run_bass_kernel_spmd
from concourse.masks import make_identity
from concourse.tile import TileContext, ScopedClock

# ----------------------------------------------------------------------------
# Problem constants (hardcoded per contract)
# ----------------------------------------------------------------------------
CUTOFF = 5.0
MAX_BASIS_DEG = 2
MAX_DEG = 4
N_ATOMS = 20000
N_EDGES = 50000
F = 16
N_CORES = 8

NSH = (MAX_BASIS_DEG + 1) ** 2        # 9 spherical-harmonic components
NB = (MAX_DEG + 1) ** 2               # 25 atom-feature m-slots
BPAD = 26                             # b padded for alignment
NC_OUT = 2 * NB                       # 50 output (parity, c) slots
ABLK = NSH * BPAD                     # 234 Z-columns per f
KUSE = 2 * ABLK                       # 468 useful Z-columns per f-pair
FPBLK = 512                           # padded Z-columns per f-pair
ZCOLS = 8 * FPBLK                     # 4096
EPC = 6400                            # edges per core (padded from 6250)
EBLK = 128                            # edges per block
NBLK = EPC // EBLK                    # 50 blocks per core
SUPER = 4                             # e-blocks per superblock
NCHUNK = ZCOLS // 128                 # 32 zT chunks (4 per f-pair)
GROW = F * BPAD                       # gather row (bf16)
NSB_C = (NBLK + SUPER - 1) // SUPER   # superblocks per core

PATHS = [(l1, l2, l3)
         for l1 in range(MAX_BASIS_DEG + 1)
         for l2 in range(MAX_DEG + 1)
         for l3 in range(abs(l1 - l2), min(l1 + l2, MAX_DEG) + 1)]

ZDT = mybir.dt.bfloat16               # Z / zT / W dtype


# ----------------------------------------------------------------------------
# Clebsch-Gordan coefficients (host, numpy only)
# ----------------------------------------------------------------------------
def _fac(n):
    return math.factorial(n)


def _cg(j1, m1, j2, m2, j3, m3):
    if m1 + m2 != m3:
        return 0.0
    if j3 < abs(j1 - j2) or j3 > j1 + j2:
        return 0.0
    pre = math.sqrt((2 * j3 + 1) * _fac(j3 + j1 - j2) * _fac(j3 - j1 + j2)
                    * _fac(j1 + j2 - j3) / _fac(j1 + j2 + j3 + 1))
    pre *= math.sqrt(_fac(j3 + m3) * _fac(j3 - m3) * _fac(j1 - m1) * _fac(j1 + m1)
                     * _fac(j2 - m2) * _fac(j2 + m2))
    s = 0.0
    for k in range(max(0, j2 - j3 - m1, j1 - j3 + m2),
                   min(j1 + j2 - j3, j1 - m1, j2 + m2) + 1):
        s += (-1) ** k / (_fac(k) * _fac(j1 + j2 - j3 - k) * _fac(j1 - m1 - k)
                          * _fac(j2 + m2 - k) * _fac(j3 - j2 + m1 + k)
                          * _fac(j3 - j1 - m2 + k))
    return pre * s


def _umat(l):
    U = np.zeros((2 * l + 1, 2 * l + 1), dtype=np.complex128)
    s2 = 1.0 / np.sqrt(2.0)
    for m in range(-l, l + 1):
        if m > 0:
            U[m + l, m + l] = ((-1) ** m) * s2
            U[m + l, -m + l] = s2
        elif m == 0:
            U[l, l] = 1.0
        else:
            am = -m
            U[m + l, m + l] = 1j * s2
            U[m + l, am + l] = -1j * ((-1) ** am) * s2
    return U


def _real_cg(l1, l2, l3):
    C = np.zeros((2 * l1 + 1, 2 * l2 + 1, 2 * l3 + 1), dtype=np.complex128)
    for m1 in range(-l1, l1 + 1):
        for m2 in range(-l2, l2 + 1):
            m3 = m1 + m2
            if -l3 <= m3 <= l3:
                C[m1 + l1, m2 + l2, m3 + l3] = _cg(l1, m1, l2, m2, l3, m3)
    G = np.einsum('aA,bB,cC,ABC->abc', _umat(l1), _umat(l2),
                  np.conj(_umat(l3)), C)
    G = G.real if (l1 + l2 + l3) % 2 == 0 else G.imag
    return np.ascontiguousarray(G)


def _build_weight_tensor(tp_weights):
    """W -> [8*FPBLK, 2*NC_OUT] in Z-column order (fp, df, a, b-padded),
    entry = CG[a,b,c] * tp_weights[path, f], f = 2*fp+df, output column
    q = 2*c + df."""
    G_abc = np.zeros((NSH, NB, NC_OUT), dtype=np.float64)
    for p, (l1, l2, l3) in enumerate(PATHS):
        G = _real_cg(l1, l2, l3)
        par = (l1 + l2 + l3) % 2
        for ai in range(2 * l1 + 1):
            for bi in range(2 * l2 + 1):
                for ci in range(2 * l3 + 1):
                    v = G[ai, bi, ci]
                    if v != 0.0:
                        ga = l1 * l1 + ai
                        gb = l2 * l2 + bi
                        gc = par * NB + l3 * l3 + ci
                        G_abc[ga, gb, gc] = v
    path_idx = {}
    for p, (l1, l2, l3) in enumerate(PATHS):
        path_idx[(l1, l2, l3)] = p
    l_of_a = [0, 1, 1, 1, 2, 2, 2, 2, 2]
    l_of_b = [int(np.sqrt(b)) for b in range(NB)]
    l_of_c = [int(np.sqrt(c % NB)) for c in range(NC_OUT)]

    W = np.zeros((F, NSH, BPAD, NC_OUT), dtype=np.float64)
    for ga in range(NSH):
        for gb in range(NB):
            nz = np.nonzero(G_abc[ga, gb])[0]
            if len(nz) == 0:
                continue
            for gc in nz:
                p = path_idx[(l_of_a[ga], l_of_b[gb], l_of_c[gc])]
                for f in range(F):
                    W[f, ga, gb, gc] = G_abc[ga, gb, gc] * float(tp_weights[p, f])
    W = W.reshape(F, ABLK, NC_OUT)
    out = np.zeros((8, FPBLK, 2 * NC_OUT), dtype=np.float64)
    for fp in range(8):
        for df in range(2):
            out[fp, df * ABLK:(df + 1) * ABLK, df::2] = W[2 * fp + df]
    return out.reshape(8 * FPBLK, 2 * NC_OUT)


# ----------------------------------------------------------------------------
# Device kernel builder
# ----------------------------------------------------------------------------
_NC_CACHE = {}


def _drain_and_barrier_patched(self, tick_clock, wait_clock):
    # this container's walrus supports only one sync-wait per CTRL
    nc = self.nc
    drain_inst = nc.sync.drain()
    wait_clock.add_sem_waits(drain_inst.ins,
                             ScopedClock({None: tick_clock.global_clock}))
    si = drain_inst.ins.sync_info
    waits = list(si.on_wait) if si else []
    if len(waits) > 1:
        drain_inst.ins.sync_info = mybir.SyncInfo(on_wait=[waits[0]],
                                                  on_update=list(si.on_update))
        for w in waits[1:]:
            d2 = nc.sync.drain()
            d2.ins.sync_info = mybir.SyncInfo(on_wait=[w], on_update=[])
    nc.all_engine_barrier()
    assert self.sems is not None
    popped = nc._tile_sem_poison_stack.pop()
    assert popped is self._sem_poison
    nc.clear_and_free_semaphores(list(self.sems.allocated().values()))
    nc.all_engine_barrier()


TileContext._drain_and_barrier = _drain_and_barrier_patched

# per-f-pair contraction pieces: (chunk, rows); last piece only 84 useful rows
_PIECES = [(0, 128), (1, 128), (2, 128), (3, KUSE - 3 * 128)]

# engine assignment knobs ('v' = DVE, 'a' = Act, 'p' = Pool/gpsimd)
# NOTE: gpsimd cannot read PSUM, so BPAIR/OG must use only 'v'/'a'
ZSLICE_ENG = ['v', 'v', 'v', 'v', 'v', 'v', 'a', 'a', 'p']
BPAIR_ENG = ['v', 'a', 'v', 'a', 'v', 'a', 'v', 'v',
             'v', 'a', 'v', 'a', 'v', 'a', 'v', 'v']
OG_ENG = ['a', 'v', 'a', 'v']


def _split_multi_waits(nc):
    """This container's walrus supports one sync-wait per instruction; move
    extra waits onto injected same-engine NoOps placed just before."""
    for f in nc.m.functions:
        for bb in f.blocks:
            newl = []
            changed = False
            for inst in bb.instructions:
                si = inst.sync_info
                waits = list(si.on_wait) if si else []
                if len(waits) > 1:
                    changed = True
                    for k, w in enumerate(waits[:-1]):
                        nop = mybir.InstDrain(name=f"{inst.name}-sw{k}",
                                              ins=[], outs=[])
                        nop.engine = inst.engine
                        nop.sync_info = mybir.SyncInfo(on_wait=[w], on_update=[])
                        newl.append(nop)
                    inst.sync_info = mybir.SyncInfo(on_wait=[waits[-1]],
                                                    on_update=list(si.on_update))
                newl.append(inst)
            if changed:
                bb.instructions = newl


def _build_bass(split_waits=True):
    nc = bass.Bass("TRN2", target_bir_lowering=False, debug=False)
    dt = mybir.dt
    f32 = dt.float32
    i32 = dt.int32
    zdt = ZDT
    AF = mybir.ActivationFunctionType
    OP = mybir.AluOpType

    a2 = nc.dram_tensor("a2", [N_ATOMS, GROW], ZDT, kind="ExternalInput").ap()
    idx = nc.dram_tensor("idx", [EPC * 2], i32, kind="ExternalInput").ap()
    disp = nc.dram_tensor("disp", [EPC * 4], f32, kind="ExternalInput").ap()
    wmat = nc.dram_tensor("wmat", [8 * FPBLK, 2 * NC_OUT], ZDT,
                          kind="ExternalInput").ap()
    out2 = nc.dram_tensor("out2", [2 * NC_OUT * 8 * EPC], f32,
                          kind="ExternalOutput").ap()

    NSB = NSB_C
    c1 = 0.4886025119029199
    c2 = 1.0925484305920792
    MAGICI = 0x5f3759df
    MAGICF = 8388608.0

    from contextlib import ExitStack
    with TileContext(nc) as tc, ExitStack() as ctx:
        consts = ctx.enter_context(tc.tile_pool(name="consts", bufs=1))
        wpool = ctx.enter_context(tc.tile_pool(name="wpool", bufs=1))
        epool = ctx.enter_context(tc.tile_pool(name="epool", bufs=2))   # y gather
        ypp = ctx.enter_context(tc.tile_pool(name="ypp", bufs=3))       # yp tiles
        spool = ctx.enter_context(tc.tile_pool(name="spool", bufs=2))   # geometry
        zpool = ctx.enter_context(tc.tile_pool(name="zpool", bufs=2))   # Z tiles
        ztp = ctx.enter_context(tc.tile_pool(name="ztp", bufs=2))       # zT sbuf
        opool = ctx.enter_context(tc.tile_pool(name="opool", bufs=1))   # og sbuf
        pst = ctx.enter_context(tc.tile_pool(name="pst", bufs=4, space="PSUM"))
        psm = ctx.enter_context(tc.tile_pool(name="psm", bufs=2, space="PSUM"))

        # ---- constants ----
        identb = consts.tile([128, 128], zdt)
        make_identity(nc, identb[:])
        biasC2 = consts.tile([128, 1], f32)
        nc.vector.memset(biasC2[:], CUTOFF * CUTOFF)
        # krc4[p, ebi, f] = (f+1)/CUTOFF for ebi in 0..3
        kint = consts.tile([128, F], i32)
        nc.gpsimd.iota(kint[:], pattern=[[1, F]], base=1, channel_multiplier=0)
        krc4 = consts.tile([128, SUPER * F], f32)
        for s_ in range(SUPER):
            nc.vector.tensor_copy(out=krc4[:, s_ * F:(s_ + 1) * F],
                                  in_=kint[:])
        nc.vector.tensor_scalar(out=krc4[:], in0=krc4[:], scalar1=1.0 / CUTOFF,
                                scalar2=None, op0=OP.mult)

        # stationary W: one [128, 32*100] tile, one DMA. Block (4*fp+pi)
        # holds piece rows [fp*512+pi*128, +128) (reading into the zero pad
        # past row 468 is harmless; matmul reads only the useful rows).
        wt_all = consts.tile([128, 32 * 2 * NC_OUT], zdt)
        nc.sync.dma_start(
            out=wt_all[:],
            in_=AP(wmat.tensor, 0,
                   [[2 * NC_OUT, 128], [128 * 2 * NC_OUT, 32], [1, 2 * NC_OUT]]))

        def wt(fp, pi):
            blk = 4 * fp + pi
            rows = _PIECES[pi][1]
            return wt_all[:rows, blk * 2 * NC_OUT:(blk + 1) * 2 * NC_OUT]

        for sb in range(NSB):
            eblocks = list(range(sb * SUPER, min((sb + 1) * SUPER, NBLK)))
            nebs = len(eblocks)
            ne = nebs * 128
            e0 = sb * SUPER * EBLK

            # ---- load idx/disp for the superblock ----
            idx_t = spool.tile([128, SUPER, 2], i32, tag="idxt", name="idxt")
            nc.sync.dma_start(
                out=idx_t[:, :nebs, :],
                in_=AP(idx.tensor, e0 * 2,
                       [[2, 128], [256, nebs], [1, 2]]))
            d4 = spool.tile([128, SUPER, 4], f32, tag="d4")
            nc.sync.dma_start(
                out=d4[:, :nebs, :],
                in_=AP(disp.tensor, e0 * 4,
                       [[4, 128], [512, nebs], [1, 4]]))

            # ---- per-eblock endpoint gathers (single-offset-per-partition
            # indirect DMA, baseline-proven), second endpoint added in-DMA ----
            y2 = epool.tile([128, SUPER * GROW], zdt, tag="y2")
            y2ap = y2[:]
            yv = y2ap.rearrange("p (s w) -> p s w", s=SUPER)
            for ebi in range(nebs):
                nc.gpsimd.indirect_dma_start(
                    out=yv[:, ebi, :], out_offset=None, in_=a2[:],
                    in_offset=bass.IndirectOffsetOnAxis(
                        ap=idx_t[:, ebi, 0:1], axis=0))
                nc.gpsimd.indirect_dma_start(
                    out=yv[:, ebi, :], out_offset=None, in_=a2[:],
                    in_offset=bass.IndirectOffsetOnAxis(
                        ap=idx_t[:, ebi, 1:2], axis=0),
                    compute_op=OP.add)

            # ---- geometry for the whole superblock ([128, nebs*k] tiles) ----
            def st(k, name, dtype=f32):
                return spool.tile([128, SUPER, k], dtype, tag=name, name=name)

            sq = st(3, "sq")
            nc.scalar.square(sq[:, :nebs, :], d4[:, :nebs, 0:3])
            r2 = st(1, "r2")
            nc.vector.tensor_reduce(out=r2[:, :nebs, :], in_=sq[:, :nebs, :],
                                    op=OP.add, axis=mybir.AxisListType.X)
            # rsqrt via bit trick + 2 Newton steps (keeps sqrt off ActE)
            yi = st(1, "yi", i32)
            nc.vector.tensor_scalar(out=yi[:, :nebs, :],
                                    in0=r2[:, :nebs, :].bitcast(i32),
                                    scalar1=1, scalar2=None,
                                    op0=OP.logical_shift_right)
            nc.vector.tensor_scalar(out=yi[:, :nebs, :], in0=yi[:, :nebs, :],
                                    scalar1=-1, scalar2=None,
                                    op0=OP.bitwise_xor)
            nc.vector.tensor_scalar(out=yi[:, :nebs, :], in0=yi[:, :nebs, :],
                                    scalar1=MAGICI + 1, scalar2=None,
                                    op0=OP.add)
            rv = st(1, "rv")
            nc.vector.tensor_copy(out=rv[:, :nebs, :],
                                  in_=yi[:, :nebs, :].bitcast(f32))
            nt = st(1, "nt")
            for _ in range(2):
                nc.vector.tensor_tensor(out=nt[:, :nebs, :], in0=r2[:, :nebs, :],
                                        in1=rv[:, :nebs, :], op=OP.mult)
                nc.vector.tensor_tensor(out=nt[:, :nebs, :], in0=nt[:, :nebs, :],
                                        in1=rv[:, :nebs, :], op=OP.mult)
                nc.vector.tensor_scalar(out=nt[:, :nebs, :], in0=nt[:, :nebs, :],
                                        scalar1=-0.5, scalar2=1.5,
                                        op0=OP.mult, op1=OP.add)
                nc.vector.tensor_tensor(out=rv[:, :nebs, :], in0=rv[:, :nebs, :],
                                        in1=nt[:, :nebs, :], op=OP.mult)
            r = st(1, "r")
            nc.vector.tensor_tensor(out=r[:, :nebs, :], in0=r2[:, :nebs, :],
                                    in1=rv[:, :nebs, :], op=OP.mult)
            # mask = (r2 < C^2) via sign
            msgn = st(1, "msgn")
            nc.scalar.activation(msgn[:, :nebs, :], r2[:, :nebs, :], AF.Sign,
                                 bias=biasC2[:, 0:1], scale=-1.0)
            mask = st(1, "mask")
            nc.vector.tensor_scalar(out=mask[:, :nebs, :], in0=msgn[:, :nebs, :],
                                    scalar1=0.5, scalar2=0.5,
                                    op0=OP.mult, op1=OP.add)
            # unit displacement, component-major [128, 3, SUPER] so Pool ops
            # downstream see unit-stride last dims (Pool codegen requirement)
            u3 = spool.tile([128, 3, SUPER], f32, tag="u3", name="u3")
            d4ap = d4[:]
            rvap = rv[:]
            nc.vector.tensor_tensor(
                out=u3[:, :, :nebs],
                in0=AP(d4ap.tensor, d4ap.offset,
                       [list(d4ap.ap[0]), [1, 3], [4, nebs]]),
                in1=AP(rvap.tensor, rvap.offset,
                       [list(rvap.ap[0]), [0, 3], [1, nebs]]),
                op=OP.mult)
            ux, uy, uz = (u3[:, j, :nebs] for j in range(3))

            # spherical harmonics sh[p, a, ebi] (a-major for contiguity)
            sh = spool.tile([128, NSH, SUPER], f32, tag="sh", name="sh")
            nc.vector.memset(sh[:, 0, :nebs], 0.28209479177387814)
            nc.vector.tensor_scalar(out=sh[:, 1, :nebs], in0=uy, scalar1=c1,
                                    scalar2=None, op0=OP.mult)
            nc.vector.tensor_scalar(out=sh[:, 2, :nebs], in0=uz, scalar1=c1,
                                    scalar2=None, op0=OP.mult)
            nc.vector.tensor_scalar(out=sh[:, 3, :nebs], in0=ux, scalar1=c1,
                                    scalar2=None, op0=OP.mult)
            nc.vector.scalar_tensor_tensor(out=sh[:, 4, :nebs], in0=ux,
                                           scalar=c2, in1=uy,
                                           op0=OP.mult, op1=OP.mult)
            nc.vector.scalar_tensor_tensor(out=sh[:, 5, :nebs], in0=uy,
                                           scalar=c2, in1=uz,
                                           op0=OP.mult, op1=OP.mult)
            t6 = st(1, "t6")
            nc.vector.tensor_tensor(out=t6[:, :nebs, 0], in0=uz,
                                    in1=uz, op=OP.mult)
            nc.vector.tensor_scalar(out=sh[:, 6, :nebs], in0=t6[:, :nebs, 0],
                                    scalar1=3.0 * 0.31539156525252005,
                                    scalar2=-0.31539156525252005,
                                    op0=OP.mult, op1=OP.add)
            nc.vector.scalar_tensor_tensor(out=sh[:, 7, :nebs], in0=ux,
                                           scalar=c2, in1=uz,
                                           op0=OP.mult, op1=OP.mult)
            t8 = st(1, "t8")
            nc.vector.scalar_tensor_tensor(out=t8[:, :nebs, 0], in0=ux,
                                           scalar=0.5 * c2, in1=ux,
                                           op0=OP.mult, op1=OP.mult)
            t8b = st(1, "t8b")
            nc.vector.scalar_tensor_tensor(out=t8b[:, :nebs, 0], in0=uy,
                                           scalar=-0.5 * c2, in1=uy,
                                           op0=OP.mult, op1=OP.mult)
            nc.vector.tensor_tensor(out=sh[:, 8, :nebs], in0=t8[:, :nebs, 0],
                                    in1=t8b[:, :nebs, 0], op=OP.add)

            # radial basis rad[p, ebi, f] = sinc(k*r/C) * (r<C)
            x = st(F, "x")
            nc.vector.tensor_tensor(
                out=x[:, :nebs, :],
                in0=krc4[:].rearrange("p (s f) -> p s f", s=SUPER)[:, :nebs, :],
                in1=r[:, :nebs, :].to_broadcast([128, nebs, F]), op=OP.mult)
            px = st(F, "px")
            nc.vector.tensor_scalar(out=px[:, :nebs, :], in0=x[:, :nebs, :],
                                    scalar1=math.pi, scalar2=None, op0=OP.mult)
            prec = st(F, "prec")
            nc.vector.reciprocal(prec[:, :nebs, :], px[:, :nebs, :])
            th = st(F, "th")
            nc.vector.tensor_scalar(out=th[:, :nebs, :], in0=x[:, :nebs, :],
                                    scalar1=0.5, scalar2=MAGICF,
                                    op0=OP.mult, op1=OP.add)
            nc.vector.tensor_scalar(out=th[:, :nebs, :], in0=th[:, :nebs, :],
                                    scalar1=-MAGICF, scalar2=None, op0=OP.add)
            q = st(F, "q")
            nc.vector.scalar_tensor_tensor(out=q[:, :nebs, :],
                                           in0=th[:, :nebs, :], scalar=-2.0,
                                           in1=x[:, :nebs, :],
                                           op0=OP.mult, op1=OP.add)
            sins = st(F, "sins")
            nc.scalar.activation(sins[:, :nebs, :], q[:, :nebs, :], AF.Sin,
                                 bias=0.0, scale=math.pi)
            rad = st(F, "rad")
            nc.vector.tensor_tensor(out=rad[:, :nebs, :], in0=sins[:, :nebs, :],
                                    in1=prec[:, :nebs, :], op=OP.mult)
            nc.gpsimd.tensor_tensor(
                out=rad[:, :nebs, :], in0=rad[:, :nebs, :],
                in1=mask[:, :nebs, :].to_broadcast([128, nebs, F]), op=OP.mult)

            # ---- yp = y*rad for the whole superblock (one DVE instr) ----
            yp = ypp.tile([128, SUPER * F * BPAD], zdt, tag="yp")
            nc.gpsimd.tensor_tensor(
                out=yp[:].rearrange("p (s f b) -> p s f b", s=SUPER, f=F)[
                    :, :nebs, :, :],
                in0=AP(y2ap.tensor, y2ap.offset,
                       [list(y2ap.ap[0]), [GROW, nebs], [BPAD, F], [1, BPAD]]),
                in1=rad[:, :nebs, :, None].to_broadcast([128, nebs, F, BPAD]),
                op=OP.mult)

            # ---- Z slices ----
            zs = []
            for ebi in range(nebs):
                z = zpool.tile([128, ZCOLS], zdt, tag=f"z{ebi}", name=f"z{ebi}")
                zs.append(z)
                zap = z[:]
                ypap = yp[:]
                for a in range(NSH):
                    zsl = AP(zap.tensor, zap.offset + a * BPAD,
                             [list(zap.ap[0]), [FPBLK, 8], [ABLK, 2], [1, BPAD]])
                    ysl = AP(ypap.tensor, ypap.offset + ebi * F * BPAD,
                             [list(ypap.ap[0]), [2 * BPAD, 8], [BPAD, 2],
                              [1, BPAD]])
                    sca = sh[:, a, ebi:ebi + 1]
                    eng = ZSLICE_ENG[a]
                    if eng == 'v':
                        nc.vector.tensor_scalar(out=zsl, in0=ysl, scalar1=sca,
                                                scalar2=None, op0=OP.mult)
                    elif eng == 'a':
                        nc.scalar.activation(zsl, ysl, AF.Copy,
                                             bias=0.0, scale=sca)
                    else:
                        nc.gpsimd.tensor_scalar(out=zsl, in0=ysl, scalar1=sca,
                                                scalar2=None, op0=OP.mult)

            # ---- per fp-pair-group: transpose its 4 chunk-pairs, then its
            # matmuls; interleaving keeps PE busy while copies drain ----
            zts = {}
            for fpp in range(4):
                for pr in range(4 * fpp, 4 * fpp + 4):
                    zt = ztp.tile([128, 1024], zdt, tag=f"zt_{pr}",
                                  name=f"zt_{pr}")
                    zts[pr] = zt
                    pt = pst.tile([128, 1024], zdt, tag="pt", space="PSUM")
                    for half in range(2):
                        c = 2 * pr + half
                        rows = _PIECES[c % 4][1]
                        for ebi in range(nebs):
                            nc.tensor.transpose(
                                out=pt[:rows,
                                       half * 512 + ebi * 128:
                                       half * 512 + (ebi + 1) * 128],
                                in_=zs[ebi][:, c * 128:c * 128 + rows],
                                identity=identb[:])
                    eng = BPAIR_ENG[pr]
                    if eng == 'v':
                        nc.vector.tensor_copy(out=zt[:], in_=pt[:])
                    elif eng == 'a':
                        nc.scalar.copy(out=zt[:], in_=pt[:])
                    else:
                        nc.gpsimd.tensor_copy(out=zt[:], in_=pt[:])

                po = psm.tile([2 * NC_OUT, 1024], f32, tag="po", space="PSUM")
                for half in range(2):
                    fp = 2 * fpp + half
                    for pi, (ch, rows) in enumerate(_PIECES):
                        c = 4 * fp + ch
                        nc.tensor.matmul(
                            out=po[:, half * 512:half * 512 + ne],
                            lhsT=wt(fp, pi),
                            rhs=zts[c // 2][:rows,
                                            (c % 2) * 512:(c % 2) * 512 + ne],
                            start=(pi == 0), stop=(pi == len(_PIECES) - 1))
                og = opool.tile([2 * NC_OUT, 1024], f32, tag=f"og{fpp}",
                                name=f"og{fpp}")
                eng = OG_ENG[fpp]
                if eng == 'v':
                    nc.vector.tensor_copy(out=og[:], in_=po[:])
                else:
                    nc.scalar.copy(out=og[:], in_=po[:])
                ogap = og[:]
                nc.sync.dma_start(
                    out=AP(out2.tensor,
                           fpp * 200 * EPC + sb * SUPER * 128,
                           [[EPC, 2 * NC_OUT], [100 * EPC, 2], [1, ne]]),
                    in_=AP(ogap.tensor, ogap.offset,
                           [list(ogap.ap[0]), [512, 2], [1, ne]]))

    if split_waits:
        _split_multi_waits(nc)
    return nc


def _get_nc():
    if "nc" not in _NC_CACHE:
        _NC_CACHE["nc"] = _build_bass()
    return _NC_CACHE["nc"]


# ----------------------------------------------------------------------------
# Host entry point
# ----------------------------------------------------------------------------
def kernel(atomic_descriptors, tp_weights, neighbour_displacements,
           neighbour_indices):
    atomic_descriptors = np.asarray(atomic_descriptors, dtype=np.float32)
    tp_weights = np.asarray(tp_weights, dtype=np.float32)
    neighbour_displacements = np.asarray(neighbour_displacements, dtype=np.float32)
    neighbour_indices = np.asarray(neighbour_indices, dtype=np.int32)

    # relayout atom table: (A, 1, 25, 16) -> (A, 16, 26) f-major, b padded
    A = atomic_descriptors.reshape(N_ATOMS, NB, F)
    import ml_dtypes
    a2 = np.zeros((N_ATOMS, F, BPAD), dtype=np.float32)
    a2[:, :, :NB] = A.transpose(0, 2, 1)
    a2 = a2.reshape(N_ATOMS, F * BPAD).astype(ml_dtypes.bfloat16)

    import ml_dtypes
    wmat = _build_weight_tensor(tp_weights).astype(ml_dtypes.bfloat16)

    in_maps = []
    shard = N_EDGES // N_CORES
    for c in range(N_CORES):
        idxp = np.zeros((EPC, 2), dtype=np.int32)
        dispp = np.zeros((EPC, 4), dtype=np.float32)
        idxp[:shard] = neighbour_indices[c * shard:(c + 1) * shard]
        d = neighbour_displacements[c * shard:(c + 1) * shard]
        dispp[:shard, :3] = d
        dispp[shard:, :3] = 1.0  # harmless dummy
        in_maps.append({"a2": a2, "idx": idxp.reshape(-1),
                        "disp": dispp.reshape(-1), "wmat": wmat})

    global _last_in_maps
    _last_in_maps = in_maps
    nc = _get_nc()
    res = run_bass_kernel_spmd(nc, in_maps, core_ids=list(range(N_CORES)))

    out = np.empty((N_EDGES, 2, NB, F), dtype=np.float32)
    for c in range(N_CORES):
        o2 = res.results[c]["out2"].reshape(2 * NC_OUT * 8, EPC)
        # row = fp*100 + 2*(par*25+m) + df ; f = 2*fp + df
        o = o2.reshape(8, NC_OUT, 2, EPC)[:, :, :, :shard]
        o = o.reshape(8, 2, NB, 2, shard)          # (fp, par, m, df, e)
        o = o.transpose(4, 1, 2, 0, 3)             # (e, par, m, fp, df)
        out[c * shard:(c + 1) * shard] = o.reshape(shard, 2, NB, F)
    return out


if __name__ == "__main__":
    rng = np.random.default_rng(0)
    inputs = {
        "atomic_descriptors": rng.standard_normal((N_ATOMS, 1, NB, F), dtype=np.float32),
        "tp_weights": (rng.standard_normal((len(PATHS), F)) * 0.1).astype(np.float32),
        "neighbour_displacements": (rng.standard_normal((N_EDGES, 3)) * 1.5).astype(np.float32),
        "neighbour_indices": rng.integers(0, N_ATOMS, (N_EDGES, 2)).astype(np.int32),
    }
    out = kernel(**inputs)
    print("kernel ran, out shape", out.shape)


# revision 48
# speedup vs baseline: 1.7189x; 1.0068x over previous
"""Bond-centered tensor-moment descriptor kernel for Trainium2 (8 NeuronCores).

Strategy: edges are sharded 8 ways; every core gets the full (relaid-out)
atom-descriptor table and gathers its edge endpoints with indirect DMA
(one indirect DMA per endpoint per 4-block superblock, summed in-DMA).
The Clebsch-Gordan tensor product is computed as: build Z[e,(fp,df,a,b)] =
sh_a(u)*rad_f(r)*y_b,f with per-partition-scalar ops, transpose Z to
feature-partitions with TensorE, then one accumulating matmul chain per
f-pair whose stationary weights fold CG coefficients and tp_weights.
The [100, e] matmul results are DMAed to DRAM untransposed; the host
does the final (free) relayout to [E, 2, 25, 16].
"""
import math
import numpy as np

import concourse.bass as bass
import concourse.tile as tile
from concourse import mybir
from concourse.bass import AP
from concourse.bass_utils import run_bass_kernel_spmd
from concourse.masks import make_identity
from concourse.tile import TileContext, ScopedClock

# ----------------------------------------------------------------------------
# Problem constants (hardcoded per contract)
# ----------------------------------------------------------------------------
CUTOFF = 5.0
MAX_BASIS_DEG = 2
MAX_DEG = 4
N_ATOMS = 20000
N_EDGES = 50000
F = 16
N_CORES = 8

NSH = (MAX_BASIS_DEG + 1) ** 2        # 9 spherical-harmonic components
NB = (MAX_DEG + 1) ** 2               # 25 atom-feature m-slots
BPAD = 26                             # b padded for alignment
NC_OUT = 2 * NB                       # 50 output (parity, c) slots
ABLK = NSH * BPAD                     # 234 Z-columns per f
KUSE = 2 * ABLK                       # 468 useful Z-columns per f-pair
FPBLK = 512                           # padded Z-columns per f-pair
ZCOLS = 8 * FPBLK                     # 4096
EPC = 6400                            # edges per core (padded from 6250)
EBLK = 128                            # edges per block
NBLK = EPC // EBLK                    # 50 blocks per core
SUPER = 4                             # e-blocks per superblock
NCHUNK = ZCOLS // 128                 # 32 zT chunks (4 per f-pair)
GROW = F * BPAD                       # gather row (bf16)
NSB_C = (NBLK + SUPER - 1) // SUPER   # superblocks per core

PATHS = [(l1, l2, l3)
         for l1 in range(MAX_BASIS_DEG + 1)
         for l2 in range(MAX_DEG + 1)
         for l3 in range(abs(l1 - l2), min(l1 + l2, MAX_DEG) + 1)]

ZDT = mybir.dt.bfloat16               # Z / zT / W dtype


# ----------------------------------------------------------------------------
# Clebsch-Gordan coefficients (host, numpy only)
# ----------------------------------------------------------------------------
def _fac(n):
    return math.factorial(n)


def _cg(j1, m1, j2, m2, j3, m3):
    if m1 + m2 != m3:
        return 0.0
    if j3 < abs(j1 - j2) or j3 > j1 + j2:
        return 0.0
    pre = math.sqrt((2 * j3 + 1) * _fac(j3 + j1 - j2) * _fac(j3 - j1 + j2)
                    * _fac(j1 + j2 - j3) / _fac(j1 + j2 + j3 + 1))
    pre *= math.sqrt(_fac(j3 + m3) * _fac(j3 - m3) * _fac(j1 - m1) * _fac(j1 + m1)
                     * _fac(j2 - m2) * _fac(j2 + m2))
    s = 0.0
    for k in range(max(0, j2 - j3 - m1, j1 - j3 + m2),
                   min(j1 + j2 - j3, j1 - m1, j2 + m2) + 1):
        s += (-1) ** k / (_fac(k) * _fac(j1 + j2 - j3 - k) * _fac(j1 - m1 - k)
                          * _fac(j2 + m2 - k) * _fac(j3 - j2 + m1 + k)
                          * _fac(j3 - j1 - m2 + k))
    return pre * s


def _umat(l):
    U = np.zeros((2 * l + 1, 2 * l + 1), dtype=np.complex128)
    s2 = 1.0 / np.sqrt(2.0)
    for m in range(-l, l + 1):
        if m > 0:
            U[m + l, m + l] = ((-1) ** m) * s2
            U[m + l, -m + l] = s2
        elif m == 0:
            U[l, l] = 1.0
        else:
            am = -m
            U[m + l, m + l] = 1j * s2
            U[m + l, am + l] = -1j * ((-1) ** am) * s2
    return U


def _real_cg(l1, l2, l3):
    C = np.zeros((2 * l1 + 1, 2 * l2 + 1, 2 * l3 + 1), dtype=np.complex128)
    for m1 in range(-l1, l1 + 1):
        for m2 in range(-l2, l2 + 1):
            m3 = m1 + m2
            if -l3 <= m3 <= l3:
                C[m1 + l1, m2 + l2, m3 + l3] = _cg(l1, m1, l2, m2, l3, m3)
    G = np.einsum('aA,bB,cC,ABC->abc', _umat(l1), _umat(l2),
                  np.conj(_umat(l3)), C)
    G = G.real if (l1 + l2 + l3) % 2 == 0 else G.imag
    return np.ascontiguousarray(G)


def _build_weight_tensor(tp_weights):
    """W -> [8*FPBLK, 2*NC_OUT] in Z-column order (fp, df, a, b-padded),
    entry = CG[a,b,c] * tp_weights[path, f], f = 2*fp+df, output column
    q = 2*c + df."""
    G_abc = np.zeros((NSH, NB, NC_OUT), dtype=np.float64)
    for p, (l1, l2, l3) in enumerate(PATHS):
        G = _real_cg(l1, l2, l3)
        par = (l1 + l2 + l3) % 2
        for ai in range(2 * l1 + 1):
            for bi in range(2 * l2 + 1):
                for ci in range(2 * l3 + 1):
                    v = G[ai, bi, ci]
                    if v != 0.0:
                        ga = l1 * l1 + ai
                        gb = l2 * l2 + bi
                        gc = par * NB + l3 * l3 + ci
                        G_abc[ga, gb, gc] = v
    path_idx = {}
    for p, (l1, l2, l3) in enumerate(PATHS):
        path_idx[(l1, l2, l3)] = p
    l_of_a = [0, 1, 1, 1, 2, 2, 2, 2, 2]
    l_of_b = [int(np.sqrt(b)) for b in range(NB)]
    l_of_c = [int(np.sqrt(c % NB)) for c in range(NC_OUT)]

    W = np.zeros((F, NSH, BPAD, NC_OUT), dtype=np.float64)
    for ga in range(NSH):
        for gb in range(NB):
            nz = np.nonzero(G_abc[ga, gb])[0]
            if len(nz) == 0:
                continue
            for gc in nz:
                p = path_idx[(l_of_a[ga], l_of_b[gb], l_of_c[gc])]
                for f in range(F):
                    W[f, ga, gb, gc] = G_abc[ga, gb, gc] * float(tp_weights[p, f])
    W = W.reshape(F, ABLK, NC_OUT)
    out = np.zeros((8, FPBLK, 2 * NC_OUT), dtype=np.float64)
    for fp in range(8):
        for df in range(2):
            out[fp, df * ABLK:(df + 1) * ABLK, df::2] = W[2 * fp + df]
    return out.reshape(8 * FPBLK, 2 * NC_OUT)


# ----------------------------------------------------------------------------
# Device kernel builder
# ----------------------------------------------------------------------------
_NC_CACHE = {}


def _drain_and_barrier_patched(self, tick_clock, wait_clock):
    # this container's walrus supports only one sync-wait per CTRL
    nc = self.nc
    drain_inst = nc.sync.drain()
    wait_clock.add_sem_waits(drain_inst.ins,
                             ScopedClock({None: tick_clock.global_clock}))
    si = drain_inst.ins.sync_info
    waits = list(si.on_wait) if si else []
    if len(waits) > 1:
        drain_inst.ins.sync_info = mybir.SyncInfo(on_wait=[waits[0]],
                                                  on_update=list(si.on_update))
        for w in waits[1:]:
            d2 = nc.sync.drain()
            d2.ins.sync_info = mybir.SyncInfo(on_wait=[w], on_update=[])
    nc.all_engine_barrier()
    assert self.sems is not None
    popped = nc._tile_sem_poison_stack.pop()
    assert popped is self._sem_poison
    nc.clear_and_free_semaphores(list(self.sems.allocated().values()))
    nc.all_engine_barrier()


TileContext._drain_and_barrier = _drain_and_barrier_patched

# per-f-pair contraction pieces: (chunk, rows); last piece only 84 useful rows
_PIECES = [(0, 128), (1, 128), (2, 128), (3, KUSE - 3 * 128)]

# engine assignment knobs ('v' = DVE, 'a' = Act, 'p' = Pool/gpsimd)
# NOTE: gpsimd cannot read PSUM, so BPAIR/OG must use only 'v'/'a'
ZSLICE_ENG = ['v', 'v', 'v', 'v', 'v', 'v', 'a', 'a', 'p']
BPAIR_ENG = ['v', 'a', 'v', 'a', 'v', 'a', 'v', 'a',
             'v', 'a', 'v', 'a', 'v', 'a', 'v', 'a']
OG_ENG = ['a', 'v', 'a', 'v']


def _split_multi_waits(nc):
    """This container's walrus supports one sync-wait per instruction; move
    extra waits onto injected same-engine NoOps placed just before."""
    for f in nc.m.functions:
        for bb in f.blocks:
            newl = []
            changed = False
            for inst in bb.instructions:
                si = inst.sync_info
                waits = list(si.on_wait) if si else []
                if len(waits) > 1:
                    changed = True
                    for k, w in enumerate(waits[:-1]):
                        nop = mybir.InstDrain(name=f"{inst.name}-sw{k}",
                                              ins=[], outs=[])
                        nop.engine = inst.engine
                        nop.sync_info = mybir.SyncInfo(on_wait=[w], on_update=[])
                        newl.append(nop)
                    inst.sync_info = mybir.SyncInfo(on_wait=[waits[-1]],
                                                    on_update=list(si.on_update))
                newl.append(inst)
            if changed:
                bb.instructions = newl


def _build_bass(split_waits=True):
    nc = bass.Bass("TRN2", target_bir_lowering=False, debug=False)
    dt = mybir.dt
    f32 = dt.float32
    i32 = dt.int32
    zdt = ZDT
    AF = mybir.ActivationFunctionType
    OP = mybir.AluOpType

    a2 = nc.dram_tensor("a2", [N_ATOMS, GROW], ZDT, kind="ExternalInput").ap()
    idx = nc.dram_tensor("idx", [EPC * 2], i32, kind="ExternalInput").ap()
    disp = nc.dram_tensor("disp", [EPC * 4], f32, kind="ExternalInput").ap()
    wmat = nc.dram_tensor("wmat", [8 * FPBLK, 2 * NC_OUT], ZDT,
                          kind="ExternalInput").ap()
    out2 = nc.dram_tensor("out2", [2 * NC_OUT * 8 * EPC], f32,
                          kind="ExternalOutput").ap()

    NSB = NSB_C
    c1 = 0.4886025119029199
    c2 = 1.0925484305920792
    MAGICI = 0x5f3759df
    MAGICF = 8388608.0

    from contextlib import ExitStack
    with TileContext(nc) as tc, ExitStack() as ctx:
        consts = ctx.enter_context(tc.tile_pool(name="consts", bufs=1))
        wpool = ctx.enter_context(tc.tile_pool(name="wpool", bufs=1))
        epool = ctx.enter_context(tc.tile_pool(name="epool", bufs=2))   # y gather
        ypp = ctx.enter_context(tc.tile_pool(name="ypp", bufs=3))       # yp tiles
        spool = ctx.enter_context(tc.tile_pool(name="spool", bufs=2))   # geometry
        zpool = ctx.enter_context(tc.tile_pool(name="zpool", bufs=2))   # Z tiles
        ztp = ctx.enter_context(tc.tile_pool(name="ztp", bufs=2))       # zT sbuf
        opool = ctx.enter_context(tc.tile_pool(name="opool", bufs=1))   # og sbuf
        pst = ctx.enter_context(tc.tile_pool(name="pst", bufs=4, space="PSUM"))
        psm = ctx.enter_context(tc.tile_pool(name="psm", bufs=2, space="PSUM"))

        # ---- constants ----
        identb = consts.tile([128, 128], zdt)
        make_identity(nc, identb[:])
        biasC2 = consts.tile([128, 1], f32)
        nc.vector.memset(biasC2[:], CUTOFF * CUTOFF)
        # krc4[p, ebi, f] = (f+1)/CUTOFF for ebi in 0..3
        kint = consts.tile([128, F], i32)
        nc.gpsimd.iota(kint[:], pattern=[[1, F]], base=1, channel_multiplier=0)
        krc4 = consts.tile([128, SUPER * F], f32)
        for s_ in range(SUPER):
            nc.vector.tensor_copy(out=krc4[:, s_ * F:(s_ + 1) * F],
                                  in_=kint[:])
        nc.vector.tensor_scalar(out=krc4[:], in0=krc4[:], scalar1=1.0 / CUTOFF,
                                scalar2=None, op0=OP.mult)

        # stationary W: one [128, 32*100] tile, one DMA. Block (4*fp+pi)
        # holds piece rows [fp*512+pi*128, +128) (reading into the zero pad
        # past row 468 is harmless; matmul reads only the useful rows).
        wt_all = consts.tile([128, 32 * 2 * NC_OUT], zdt)
        nc.sync.dma_start(
            out=wt_all[:],
            in_=AP(wmat.tensor, 0,
                   [[2 * NC_OUT, 128], [128 * 2 * NC_OUT, 32], [1, 2 * NC_OUT]]))

        def wt(fp, pi):
            blk = 4 * fp + pi
            rows = _PIECES[pi][1]
            return wt_all[:rows, blk * 2 * NC_OUT:(blk + 1) * 2 * NC_OUT]

        for sb in range(NSB):
            eblocks = list(range(sb * SUPER, min((sb + 1) * SUPER, NBLK)))
            nebs = len(eblocks)
            ne = nebs * 128
            e0 = sb * SUPER * EBLK

            # ---- load idx/disp for the superblock ----
            idx_t = spool.tile([128, SUPER, 2], i32, tag="idxt", name="idxt")
            nc.sync.dma_start(
                out=idx_t[:, :nebs, :],
                in_=AP(idx.tensor, e0 * 2,
                       [[2, 128], [256, nebs], [1, 2]]))
            d4 = spool.tile([128, SUPER, 4], f32, tag="d4")
            nc.sync.dma_start(
                out=d4[:, :nebs, :],
                in_=AP(disp.tensor, e0 * 4,
                       [[4, 128], [512, nebs], [1, 4]]))

            # ---- per-eblock endpoint gathers (single-offset-per-partition
            # indirect DMA, baseline-proven), second endpoint added in-DMA ----
            y2 = epool.tile([128, SUPER * GROW], zdt, tag="y2")
            y2ap = y2[:]
            yv = y2ap.rearrange("p (s w) -> p s w", s=SUPER)
            for ebi in range(nebs):
                nc.gpsimd.indirect_dma_start(
                    out=yv[:, ebi, :], out_offset=None, in_=a2[:],
                    in_offset=bass.IndirectOffsetOnAxis(
                        ap=idx_t[:, ebi, 0:1], axis=0))
                nc.gpsimd.indirect_dma_start(
                    out=yv[:, ebi, :], out_offset=None, in_=a2[:],
                    in_offset=bass.IndirectOffsetOnAxis(
                        ap=idx_t[:, ebi, 1:2], axis=0),
                    compute_op=OP.add)

            # ---- geometry for the whole superblock ([128, nebs*k] tiles) ----
            def st(k, name, dtype=f32):
                return spool.tile([128, SUPER, k], dtype, tag=name, name=name)

            sq = st(3, "sq")
            nc.scalar.square(sq[:, :nebs, :], d4[:, :nebs, 0:3])
            r2 = st(1, "r2")
            nc.vector.tensor_reduce(out=r2[:, :nebs, :], in_=sq[:, :nebs, :],
                                    op=OP.add, axis=mybir.AxisListType.X)
            # rsqrt via bit trick + 2 Newton steps (keeps sqrt off ActE)
            yi = st(1, "yi", i32)
            nc.vector.tensor_scalar(out=yi[:, :nebs, :],
                                    in0=r2[:, :nebs, :].bitcast(i32),
                                    scalar1=1, scalar2=None,
                                    op0=OP.logical_shift_right)
            nc.vector.tensor_scalar(out=yi[:, :nebs, :], in0=yi[:, :nebs, :],
                                    scalar1=-1, scalar2=None,
                                    op0=OP.bitwise_xor)
            nc.vector.tensor_scalar(out=yi[:, :nebs, :], in0=yi[:, :nebs, :],
                                    scalar1=MAGICI + 1, scalar2=None,
                                    op0=OP.add)
            rv = st(1, "rv")
            nc.vector.tensor_copy(out=rv[:, :nebs, :],
                                  in_=yi[:, :nebs, :].bitcast(f32))
            nt = st(1, "nt")
            for _ in range(2):
                nc.vector.tensor_tensor(out=nt[:, :nebs, :], in0=r2[:, :nebs, :],
                                        in1=rv[:, :nebs, :], op=OP.mult)
                nc.vector.tensor_tensor(out=nt[:, :nebs, :], in0=nt[:, :nebs, :],
                                        in1=rv[:, :nebs, :], op=OP.mult)
                nc.vector.tensor_scalar(out=nt[:, :nebs, :], in0=nt[:, :nebs, :],
                                        scalar1=-0.5, scalar2=1.5,
                                        op0=OP.mult, op1=OP.add)
                nc.vector.tensor_tensor(out=rv[:, :nebs, :], in0=rv[:, :nebs, :],
                                        in1=nt[:, :nebs, :], op=OP.mult)
            r = st(1, "r")
            nc.vector.tensor_tensor(out=r[:, :nebs, :], in0=r2[:, :nebs, :],
                                    in1=rv[:, :nebs, :], op=OP.mult)
            # mask = (r2 < C^2) via sign
            msgn = st(1, "msgn")
            nc.scalar.activation(msgn[:, :nebs, :], r2[:, :nebs, :], AF.Sign,
                                 bias=biasC2[:, 0:1], scale=-1.0)
            mask = st(1, "mask")
            nc.vector.tensor_scalar(out=mask[:, :nebs, :], in0=msgn[:, :nebs, :],
                                    scalar1=0.5, scalar2=0.5,
                                    op0=OP.mult, op1=OP.add)
            # unit displacement, component-major [128, 3, SUPER] so Pool ops
            # downstream see unit-stride last dims (Pool codegen requirement)
            u3 = spool.tile([128, 3, SUPER], f32, tag="u3", name="u3")
            d4ap = d4[:]
            rvap = rv[:]
            nc.vector.tensor_tensor(
                out=u3[:, :, :nebs],
                in0=AP(d4ap.tensor, d4ap.offset,
                       [list(d4ap.ap[0]), [1, 3], [4, nebs]]),
                in1=AP(rvap.tensor, rvap.offset,
                       [list(rvap.ap[0]), [0, 3], [1, nebs]]),
                op=OP.mult)
            ux, uy, uz = (u3[:, j, :nebs] for j in range(3))

            # spherical harmonics sh[p, a, ebi] (a-major for contiguity)
            sh = spool.tile([128, NSH, SUPER], f32, tag="sh", name="sh")
            nc.vector.memset(sh[:, 0, :nebs], 0.28209479177387814)
            nc.vector.tensor_scalar(out=sh[:, 1, :nebs], in0=uy, scalar1=c1,
                                    scalar2=None, op0=OP.mult)
            nc.vector.tensor_scalar(out=sh[:, 2, :nebs], in0=uz, scalar1=c1,
                                    scalar2=None, op0=OP.mult)
            nc.vector.tensor_scalar(out=sh[:, 3, :nebs], in0=ux, scalar1=c1,
                                    scalar2=None, op0=OP.mult)
            nc.vector.scalar_tensor_tensor(out=sh[:, 4, :nebs], in0=ux,
                                           scalar=c2, in1=uy,
                                           op0=OP.mult, op1=OP.mult)
            nc.vector.scalar_tensor_tensor(out=sh[:, 5, :nebs], in0=uy,
                                           scalar=c2, in1=uz,
                                           op0=OP.mult, op1=OP.mult)
            t6 = st(1, "t6")
            nc.vector.tensor_tensor(out=t6[:, :nebs, 0], in0=uz,
                                    in1=uz, op=OP.mult)
            nc.vector.tensor_scalar(out=sh[:, 6, :nebs], in0=t6[:, :nebs, 0],
                                    scalar1=3.0 * 0.31539156525252005,
                                    scalar2=-0.31539156525252005,
                                    op0=OP.mult, op1=OP.add)
            nc.vector.scalar_tensor_tensor(out=sh[:, 7, :nebs], in0=ux,
                                           scalar=c2, in1=uz,
                                           op0=OP.mult, op1=OP.mult)
            t8 = st(1, "t8")
            nc.vector.scalar_tensor_tensor(out=t8[:, :nebs, 0], in0=ux,
                                           scalar=0.5 * c2, in1=ux,
                                           op0=OP.mult, op1=OP.mult)
            t8b = st(1, "t8b")
            nc.vector.scalar_tensor_tensor(out=t8b[:, :nebs, 0], in0=uy,
                                           scalar=-0.5 * c2, in1=uy,
                                           op0=OP.mult, op1=OP.mult)
            nc.vector.tensor_tensor(out=sh[:, 8, :nebs], in0=t8[:, :nebs, 0],
                                    in1=t8b[:, :nebs, 0], op=OP.add)

            # radial basis rad[p, ebi, f] = sinc(k*r/C) * (r<C)
            x = st(F, "x")
            nc.vector.tensor_tensor(
                out=x[:, :nebs, :],
                in0=krc4[:].rearrange("p (s f) -> p s f", s=SUPER)[:, :nebs, :],
                in1=r[:, :nebs, :].to_broadcast([128, nebs, F]), op=OP.mult)
            px = st(F, "px")
            nc.vector.tensor_scalar(out=px[:, :nebs, :], in0=x[:, :nebs, :],
                                    scalar1=math.pi, scalar2=None, op0=OP.mult)
            prec = st(F, "prec")
            nc.vector.reciprocal(prec[:, :nebs, :], px[:, :nebs, :])
            th = st(F, "th")
            nc.vector.tensor_scalar(out=th[:, :nebs, :], in0=x[:, :nebs, :],
                                    scalar1=0.5, scalar2=MAGICF,
                                    op0=OP.mult, op1=OP.add)
            nc.vector.tensor_scalar(out=th[:, :nebs, :], in0=th[:, :nebs, :],
                                    scalar1=-MAGICF, scalar2=None, op0=OP.add)
            q = st(F, "q")
            nc.vector.scalar_tensor_tensor(out=q[:, :nebs, :],
                                           in0=th[:, :nebs, :], scalar=-2.0,
                                           in1=x[:, :nebs, :],
                                           op0=OP.mult, op1=OP.add)
            sins = st(F, "sins")
            nc.scalar.activation(sins[:, :nebs, :], q[:, :nebs, :], AF.Sin,
                                 bias=0.0, scale=math.pi)
            rad = st(F, "rad")
            nc.vector.tensor_tensor(out=rad[:, :nebs, :], in0=sins[:, :nebs, :],
                                    in1=prec[:, :nebs, :], op=OP.mult)
            nc.gpsimd.tensor_tensor(
                out=rad[:, :nebs, :], in0=rad[:, :nebs, :],
                in1=mask[:, :nebs, :].to_broadcast([128, nebs, F]), op=OP.mult)

            # ---- yp = y*rad for the whole superblock (one DVE instr) ----
            yp = ypp.tile([128, SUPER * F * BPAD], zdt, tag="yp")
            nc.gpsimd.tensor_tensor(
                out=yp[:].rearrange("p (s f b) -> p s f b", s=SUPER, f=F)[
                    :, :nebs, :, :],
                in0=AP(y2ap.tensor, y2ap.offset,
                       [list(y2ap.ap[0]), [GROW, nebs], [BPAD, F], [1, BPAD]]),
                in1=rad[:, :nebs, :, None].to_broadcast([128, nebs, F, BPAD]),
                op=OP.mult)

            # ---- Z slices ----
            zs = []
            for ebi in range(nebs):
                z = zpool.tile([128, ZCOLS], zdt, tag=f"z{ebi}", name=f"z{ebi}")
                zs.append(z)
                zap = z[:]
                ypap = yp[:]
                for a in range(NSH):
                    zsl = AP(zap.tensor, zap.offset + a * BPAD,
                             [list(zap.ap[0]), [FPBLK, 8], [ABLK, 2], [1, BPAD]])
                    ysl = AP(ypap.tensor, ypap.offset + ebi * F * BPAD,
                             [list(ypap.ap[0]), [2 * BPAD, 8], [BPAD, 2],
                              [1, BPAD]])
                    sca = sh[:, a, ebi:ebi + 1]
                    eng = ZSLICE_ENG[a]
                    if eng == 'v':
                        nc.vector.tensor_scalar(out=zsl, in0=ysl, scalar1=sca,
                                                scalar2=None, op0=OP.mult)
                    elif eng == 'a':
                        nc.scalar.activation(zsl, ysl, AF.Copy,
                                             bias=0.0, scale=sca)
                    else:
                        nc.gpsimd.tensor_scalar(out=zsl, in0=ysl, scalar1=sca,
                                                scalar2=None, op0=OP.mult)

            # ---- per fp-pair-group: transpose its 4 chunk-pairs, then its
            # matmuls; interleaving keeps PE busy while copies drain ----
            zts = {}
            for fpp in range(4):
                for pr in range(4 * fpp, 4 * fpp + 4):
                    zt = ztp.tile([128, 1024], zdt, tag=f"zt_{pr}",
                                  name=f"zt_{pr}")
                    zts[pr] = zt
                    pt = pst.tile([128, 1024], zdt, tag="pt", space="PSUM")
                    for half in range(2):
                        c = 2 * pr + half
                        rows = _PIECES[c % 4][1]
                        for ebi in range(nebs):
                            nc.tensor.transpose(
                                out=pt[:rows,
                                       half * 512 + ebi * 128:
                                       half * 512 + (ebi + 1) * 128],
                                in_=zs[ebi][:, c * 128:c * 128 + rows],
                                identity=identb[:])
                    eng = BPAIR_ENG[pr]
                    if eng == 'v':
                        nc.vector.tensor_copy(out=zt[:], in_=pt[:])
                    elif eng == 'a':
                        nc.scalar.copy(out=zt[:], in_=pt[:])
                    else:
                        nc.gpsimd.tensor_copy(out=zt[:], in_=pt[:])

                po = psm.tile([2 * NC_OUT, 1024], f32, tag="po", space="PSUM")
                for half in range(2):
                    fp = 2 * fpp + half
                    for pi, (ch, rows) in enumerate(_PIECES):
                        c = 4 * fp + ch
                        nc.tensor.matmul(
                            out=po[:, half * 512:half * 512 + ne],
                            lhsT=wt(fp, pi),
                            rhs=zts[c // 2][:rows,
                                            (c % 2) * 512:(c % 2) * 512 + ne],
                            start=(pi == 0), stop=(pi == len(_PIECES) - 1))
                og = opool.tile([2 * NC_OUT, 1024], f32, tag=f"og{fpp}",
                                name=f"og{fpp}")
                eng = OG_ENG[fpp]
                if eng == 'v':
                    nc.vector.tensor_copy(out=og[:], in_=po[:])
                else:
                    nc.scalar.copy(out=og[:], in_=po[:])
                ogap = og[:]
                nc.sync.dma_start(
                    out=AP(out2.tensor,
                           fpp * 200 * EPC + sb * SUPER * 128,
                           [[EPC, 2 * NC_OUT], [100 * EPC, 2], [1, ne]]),
                    in_=AP(ogap.tensor, ogap.offset,
                           [list(ogap.ap[0]), [512, 2], [1, ne]]))

    if split_waits:
        _split_multi_waits(nc)
    return nc


def _get_nc():
    if "nc" not in _NC_CACHE:
        _NC_CACHE["nc"] = _build_bass()
    return _NC_CACHE["nc"]


# ----------------------------------------------------------------------------
# Host entry point
# ----------------------------------------------------------------------------
def kernel(atomic_descriptors, tp_weights, neighbour_displacements,
           neighbour_indices):
    atomic_descriptors = np.asarray(atomic_descriptors, dtype=np.float32)
    tp_weights = np.asarray(tp_weights, dtype=np.float32)
    neighbour_displacements = np.asarray(neighbour_displacements, dtype=np.float32)
    neighbour_indices = np.asarray(neighbour_indices, dtype=np.int32)

    # relayout atom table: (A, 1, 25, 16) -> (A, 16, 26) f-major, b padded
    A = atomic_descriptors.reshape(N_ATOMS, NB, F)
    import ml_dtypes
    a2 = np.zeros((N_ATOMS, F, BPAD), dtype=np.float32)
    a2[:, :, :NB] = A.transpose(0, 2, 1)
    a2 = a2.reshape(N_ATOMS, F * BPAD).astype(ml_dtypes.bfloat16)

    import ml_dtypes
    wmat = _build_weight_tensor(tp_weights).astype(ml_dtypes.bfloat16)

    in_maps = []
    shard = N_EDGES // N_CORES
    for c in range(N_CORES):
        idxp = np.zeros((EPC, 2), dtype=np.int32)
        dispp = np.zeros((EPC, 4), dtype=np.float32)
        idxp[:shard] = neighbour_indices[c * shard:(c + 1) * shard]
        d = neighbour_displacements[c * shard:(c + 1) * shard]
        dispp[:shard, :3] = d
        dispp[shard:, :3] = 1.0  # harmless dummy
        in_maps.append({"a2": a2, "idx": idxp.reshape(-1),
                        "disp": dispp.reshape(-1), "wmat": wmat})

    global _last_in_maps
    _last_in_maps = in_maps
    nc = _get_nc()
    res = run_bass_kernel_spmd(nc, in_maps, core_ids=list(range(N_CORES)))

    out = np.empty((N_EDGES, 2, NB, F), dtype=np.float32)
    for c in range(N_CORES):
        o2 = res.results[c]["out2"].reshape(2 * NC_OUT * 8, EPC)
        # row = fp*100 + 2*(par*25+m) + df ; f = 2*fp + df
        o = o2.reshape(8, NC_OUT, 2, EPC)[:, :, :, :shard]
        o = o.reshape(8, 2, NB, 2, shard)          # (fp, par, m, df, e)
        o = o.transpose(4, 1, 2, 0, 3)             # (e, par, m, fp, df)
        out[c * shard:(c + 1) * shard] = o.reshape(shard, 2, NB, F)
    return out


if __name__ == "__main__":
    rng = np.random.default_rng(0)
    inputs = {
        "atomic_descriptors": rng.standard_normal((N_ATOMS, 1, NB, F), dtype=np.float32),
        "tp_weights": (rng.standard_normal((len(PATHS), F)) * 0.1).astype(np.float32),
        "neighbour_displacements": (rng.standard_normal((N_EDGES, 3)) * 1.5).astype(np.float32),
        "neighbour_indices": rng.integers(0, N_ATOMS, (N_EDGES, 2)).astype(np.int32),
    }
    out = kernel(**inputs)
    print("kernel ran, out shape", out.shape)
